# revision 24
# baseline (speedup 1.0000x reference)
"""DeepReservoir (2-layer leaky ESN, T=8192, units=1024) on 8 trn2 cores.

Strategy: parallel-in-time with washout. Each core owns a contiguous
1024-step span, split into B=128 chunks of L=8 steps advancing in
lockstep as the free dimension of the recurrent matmuls. Chunks cold-
start from h=0 with W=32 washout steps (fading memory ~0.8/step).
Module 0 runs 2W washout so its recorded trajectory also covers module
1's washout window.

Precision is two-phase. Washout steps run pure bf16 (one matmul per
weight tile, fp32 PSUM); the last H+L steps run split-precision
(W ~ W_hi + W_lo, s ~ s_hi + s_lo, z ~ s_hi@W_hi + s_lo@W_hi +
s_hi@W_lo), giving ~1.6e-3 end-to-end error (validated against an
exact CPU model of this scheme).

All x/trajectory buffers use a phase-major column layout
col(t) = (t%L)*PW + t//L + PAD so that every per-step scan access is a
contiguous 128-column slice (time-major layouts force stride-L element
access, which runs ~4x slower on the DVE). The host permutes the input
projection columns to match; the trajectory and X1 projection share
one layout so the P2 matmul stays contiguous too.

Per step, matmuls and element-wise chains are interleaved over
unit-chunk groups (issue MM group g, then the DVE chain of group g-1)
because tile-framework semaphore thresholds follow program order:
issuing all matmuls before all DVE ops serializes the step and the
resulting PE idle gaps re-throttle the HAM clock gate to 1.2 GHz.
Trajectory records and output scaling run on the scalar engine (ACT),
which is otherwise idle between tanhs. State is tracked as s=2h so
the leaky blend is one fused DVE op; biases fold into the projection
matmuls via an augmented ones-row. Outputs are written to DRAM in the
on-chip layout and reordered on the host.
"""

import numpy as np

import concourse.bass as bass
import concourse.mybir as mybir
from concourse import bacc
from concourse.bass import ds
from concourse.tile import TileContext
from concourse.bass_utils import run_bass_kernel_spmd

# problem constants
T = 8192
UNITS = 1024
IN = 32
NCORES = 8
P = 128
NCH = UNITS // P  # 8 unit chunks

# tuning
W_WASH = 28           # washout steps
H_PREC = 8            # precise (split3) steps before the output window
B = 128               # time chunks per core (matmul free dim)
SPAN = T // NCORES    # 1024 steps per core
L = SPAN // B         # 8 steps per chunk
NPREC = H_PREC + L    # split3 steps per module (16)
S0 = 2 * W_WASH + L   # module-0 scan steps (64)
S1 = W_WASH + L       # module-1 scan steps (36)
PAD0 = -(-2 * W_WASH // L)  # x0 left pad in sigma units (7)
PAD1 = -(-W_WASH // L)      # x1/hb left pad (4)
PW0 = B + PAD0            # x0 cols per phase (135)
PW1 = B + PAD1            # x1/hb cols per phase (132)
X0C = L * PW0             # x0 columns (1080)
X1C = L * PW1             # x1 / hb columns (1056)
# DVE op groups over unit-chunks: pairs early, singles late (the last
# groups' add->tanh->blend chains gate the next step's matmuls)
GROUPS = [(0, 2), (2, 2), (4, 2), (6, 1), (7, 1)]

FP = mybir.dt.float32
BF = mybir.dt.bfloat16
AF = mybir.ActivationFunctionType
OP = mybir.AluOpType

_CACHE = {}


def _x0base(i):
    # contiguous col base for x0 reads at scan step i (+s for chunk s)
    return ((i - 2 * W_WASH) % L) * PW0 + (i - 2 * W_WASH) // L + PAD0


def _x1base(j):
    return ((j - W_WASH) % L) * PW1 + (j - W_WASH) // L + PAD1


def _recbase(i):
    # hb col base for the state written by mod0 step i (time 8s + i - 2W)
    return ((i - 2 * W_WASH) % L) * PW1 + (i - 2 * W_WASH) // L + PAD1


def _build():
    nc = bacc.Bacc()
    dw = {}
    for nm in ["w0hi", "w0lo", "w1hi", "w1lo", "k1hi", "k1lo"]:
        dw[nm] = nc.dram_tensor(nm, [UNITS, UNITS], BF, kind="ExternalInput")
    d_k0 = nc.dram_tensor("k0aug", [IN + 1, UNITS], FP, kind="ExternalInput")
    d_b1 = nc.dram_tensor("b1row", [1, UNITS], FP, kind="ExternalInput")
    d_u = nc.dram_tensor("u_aug", [IN + 1, X0C], FP, kind="ExternalInput")
    d_on = nc.dram_tensor("ones1", [1, X1C], FP, kind="ExternalInput")
    d_out0 = nc.dram_tensor("out0", [L, P, NCH * B], FP, kind="ExternalOutput")
    d_out1 = nc.dram_tensor("out1", [L, P, NCH * B], FP, kind="ExternalOutput")

    with TileContext(nc) as tc:
        with tc.tile_pool(name="sb", bufs=1) as pool, \
             tc.tile_pool(name="ps", bufs=1, space="PSUM") as psp:
            whi = pool.tile([P, NCH, UNITS], BF)   # W0hi, later W1hi
            wlo = pool.tile([P, NCH, UNITS], BF)   # W0lo, later W1lo
            k1hi = pool.tile([P, NCH, UNITS], BF)
            k1lo = pool.tile([P, NCH, UNITS], BF)
            k0buf = pool.tile([IN + 1, UNITS], FP)
            b1buf = pool.tile([1, UNITS], FP)
            uin = pool.tile([IN + 1, X0C], FP)
            ones1 = pool.tile([1, X1C], FP)
            xbuf = pool.tile([P, NCH, X0C], FP)    # X0x, then X1x
            hbhi = pool.tile([P, NCH, X1C], BF)    # s0 trajectory (hi)
            hblo = pool.tile([P, NCH, X1C], BF)    # s0 trajectory (lo)
            hm = [pool.tile([P, NCH, B], FP, name=f"hm{i}") for i in range(2)]
            # state hi/lo interleaved per unit-chunk: [:, c, 0:B]=hi, [:, c, B:2B]=lo
            shl = [pool.tile([P, NCH, 2 * B], BF, name=f"shl{i}") for i in range(2)]
            zg = pool.tile([P, NCH, B], FP)
            gt = pool.tile([P, NCH, B], FP)
            hout = pool.tile([P, NCH, B], FP)
            # one PSUM bank per DVE group: d-pairs in ps_s banks 0-3 (d6
            # alone in bank 3's first half... see _psl), d7 in its own bank
            # so a group's PSUM read never blocks the next group's writes
            ps_s = psp.tile([P, 8, 256], FP)       # d0-d6 (banks 0-3, slot 7 pad)
            ps_s2 = psp.tile([P, 1, 512], FP)      # d7 (bank 4)
            ps_x = psp.tile([P, 1536], FP)         # projections (banks 5-7)

            def _psl(d):
                # matmul output region for unit-chunk d
                return ps_s[:, d, 0:B] if d < 7 else ps_s2[:, 0, 0:B]

            def _psl2(d):
                # (128-col, 256-col) output regions for unit-chunk d
                if d < 7:
                    return ps_s[:, d, 0:B], ps_s[:, d, :]
                return ps_s2[:, 0, 0:B], ps_s2[:, 0, 0:2 * B]

            def _psg(g, gn):
                # DVE read region for group (g, gn), shaped [P, gn, B]
                return ps_s[:, g:g + gn, 0:B] if g < 7 else ps_s2[:, :, 0:B]

            # ---- preamble loads (scan-critical tensors first, uin split
            # across DMA queues so P0 isn't gated on one slow queue) ----
            for o in range(0, X0C, X0C // 4):
                n = min(X0C // 4, X0C - o)
                nc.sync.dma_start(out=uin[:, o:o + n], in_=d_u[:, o:o + n])
            nc.sync.dma_start(out=k0buf[:], in_=d_k0[:])
            for c in range(NCH):
                nc.sync.dma_start(out=whi[:, c, :], in_=dw["w0hi"][c * P:(c + 1) * P, :])
            for c in range(NCH):
                nc.sync.dma_start(out=wlo[:, c, :], in_=dw["w0lo"][c * P:(c + 1) * P, :])
            nc.sync.dma_start(out=b1buf[:], in_=d_b1[:])
            nc.sync.dma_start(out=ones1[:], in_=d_on[:])
            for c in range(NCH):
                nc.sync.dma_start(out=k1hi[:, c, :], in_=dw["k1hi"][c * P:(c + 1) * P, :])
                nc.sync.dma_start(out=k1lo[:, c, :], in_=dw["k1lo"][c * P:(c + 1) * P, :])
            nc.vector.memset(shl[0][:], 0.0)
            nc.vector.memset(shl[1][:, :, B:2 * B], 0.0)
            nc.vector.memset(hblo[:], 0.0)

            # ---- P0: X0x = K0aug.T @ u_aug  -> xbuf (fp32) ----
            # alternate psum buffers across d so the ACT drain of one block
            # never shares a bank with the next block's matmuls
            nt_list = [(0, 512), (512, 512), (1024, X0C - 1024)]
            for d in range(NCH):
                if d % 2 == 0:
                    for (o, n) in nt_list:
                        nc.tensor.matmul(
                            ps_x[:, o:o + n],
                            k0buf[:, d * P:(d + 1) * P],
                            uin[:, o:o + n],
                            start=True, stop=True)
                    nc.scalar.activation(xbuf[:, d, :], ps_x[:, 0:X0C], AF.Copy)
                else:
                    # scan psum banks 0-2 double as the odd-block buffer
                    for (o, n, sl) in [(0, 512, ps_s[:, 0:2, :]),
                                       (512, 512, ps_s[:, 2:4, :]),
                                       (1024, X0C - 1024, ps_s[:, 4, 0:X0C - 1024])]:
                        nc.tensor.matmul(
                            sl,
                            k0buf[:, d * P:(d + 1) * P],
                            uin[:, o:o + n],
                            start=True, stop=True)
                        nc.scalar.activation(xbuf[:, d, o:o + n], sl, AF.Copy)

            # ---- scan step (shared pipeline skeleton) ----
            # Stagger over GROUPS: emit MM(G[k]), add(G[k-1]), stt(G[k-2]),
            # post(G[k-3]); the adds run as soon as their group's matmuls
            # retire (own PSUM bank), the blend chain of the last single-
            # chunk groups finishes right behind the final matmuls.
            def run_step(mm_group, add_g, stt_g, post_g):
                ng = len(GROUPS)
                for k in range(ng + 3):
                    if k < ng:
                        mm_group(*GROUPS[k])
                    if 0 <= k - 1 < ng:
                        add_g(*GROUPS[k - 1])
                    if 0 <= k - 2 < ng:
                        stt_g(*GROUPS[k - 2])
                    if 0 <= k - 3 < ng and post_g is not None:
                        post_g(*GROUPS[k - 3])

            # ---- cheap (bf16) scan step ----
            def cheap_step(par, xb, rb=None):
                # xb: x col base (int); rb: record col base or None
                si, so = shl[par], shl[1 - par]

                def mm_group(g, gn):
                    for d in range(g, g + gn):
                        for c in range(NCH):
                            nc.tensor.matmul(
                                _psl(d), whi[:, c, d * P:(d + 1) * P],
                                si[:, c, 0:B],
                                start=(c == 0), stop=(c == NCH - 1))

                def add_g(g, gn):
                    gs = slice(g, g + gn)
                    nc.vector.tensor_tensor(
                        out=zg[:, gs, :], in0=_psg(g, gn),
                        in1=xbuf[:, gs, ds(xb, B, 1)], op=OP.add)
                    nc.scalar.activation(gt[:, gs, :], zg[:, gs, :], AF.Tanh)

                def stt_g(g, gn):
                    gs = slice(g, g + gn)
                    nc.vector.scalar_tensor_tensor(
                        out=so[:, gs, 0:B], in0=si[:, gs, 0:B], scalar=0.5,
                        in1=gt[:, gs, :], op0=OP.mult, op1=OP.add)
                    if rb is not None:
                        nc.scalar.activation(hbhi[:, gs, rb:rb + B],
                                             so[:, gs, 0:B], AF.Copy)

                run_step(mm_group, add_g, stt_g, None)

            # ---- precise (split3) scan step ----
            def prec_step(i, par, mod, first=False, out_i=None):
                hi_m, ho_m = hm[par], hm[1 - par]
                si, so = shl[par], shl[1 - par]
                xb = _x0base(i) if mod == 0 else _x1base(i)
                rb = _recbase(i) if mod == 0 else None
                if first:
                    # master state = fp32 copy of the bf16 cheap state
                    nc.vector.tensor_copy(out=hi_m[:], in_=si[:, :, 0:B])

                def mm_group(g, gn):
                    for d in range(g, g + gn):
                        psl, psl2 = _psl2(d)
                        for c in range(NCH):
                            wsl = (slice(None), c, slice(d * P, (d + 1) * P))
                            # one 256-col matmul covers s_hi and s_lo against
                            # the shared whi stationary (halves summed on DVE)
                            nc.tensor.matmul(psl2, whi[wsl], si[:, c, :],
                                             start=(c == 0), stop=False)
                            nc.tensor.matmul(psl, wlo[wsl], si[:, c, 0:B],
                                             start=False, stop=(c == NCH - 1))

                def add_g(g, gn):
                    gs = slice(g, g + gn)
                    lo_half = (ps_s[:, gs, B:2 * B] if g < 7
                               else ps_s2[:, :, B:2 * B])
                    # two adds, each with a single PSUM operand (two PSUM
                    # reads in one DVE op fail the BIR verifier)
                    nc.vector.tensor_tensor(
                        out=zg[:, gs, :], in0=_psg(g, gn),
                        in1=xbuf[:, gs, xb:xb + B], op=OP.add)
                    nc.vector.tensor_tensor(
                        out=zg[:, gs, :], in0=lo_half,
                        in1=zg[:, gs, :], op=OP.add)
                    nc.scalar.activation(gt[:, gs, :], zg[:, gs, :], AF.Tanh)

                def stt_g(g, gn):
                    gs = slice(g, g + gn)
                    nc.vector.scalar_tensor_tensor(
                        out=ho_m[:, gs, :], in0=hi_m[:, gs, :], scalar=0.5,
                        in1=gt[:, gs, :], op0=OP.mult, op1=OP.add)
                    # hi half of the bf16 split (lo follows in post)
                    nc.vector.tensor_copy(out=so[:, gs, 0:B], in_=ho_m[:, gs, :])

                def post_g(g, gn):
                    gs = slice(g, g + gn)
                    nc.vector.tensor_tensor(out=so[:, gs, B:2 * B],
                                            in0=ho_m[:, gs, :], in1=so[:, gs, 0:B],
                                            op=OP.subtract)
                    if rb is not None:
                        nc.scalar.activation(hbhi[:, gs, rb:rb + B],
                                             so[:, gs, 0:B], AF.Copy)
                        nc.scalar.activation(hblo[:, gs, rb:rb + B],
                                             so[:, gs, B:2 * B], AF.Copy)

                run_step(mm_group, add_g, stt_g, post_g)
                if out_i is not None:
                    nc.scalar.activation(hout[:], ho_m[:], AF.Copy, scale=0.5)
                    dst = d_out0 if mod == 0 else d_out1
                    nc.sync.dma_start(out=dst[out_i], in_=hout[:])

            # ---- P1: module-0 scan ----
            for i in range(0, W_WASH):
                cheap_step(i % 2, _x0base(i))
            for i in range(W_WASH, S0 - NPREC):
                cheap_step(i % 2, _x0base(i), _recbase(i))
            for i in range(S0 - NPREC, S0):
                prec_step(i, i % 2, 0, first=(i == S0 - NPREC),
                          out_i=(i - (S0 - L) if i >= S0 - L else None))

            # ---- load W1 into whi/wlo (after P1's last use) ----
            for c in range(NCH):
                nc.sync.dma_start(out=whi[:, c, :], in_=dw["w1hi"][c * P:(c + 1) * P, :])
                nc.sync.dma_start(out=wlo[:, c, :], in_=dw["w1lo"][c * P:(c + 1) * P, :])

            # ---- P2: X1x = K1h.T @ s0 + b1 (ones row) -> xbuf ----
            # x1 and hb share the phase-major layout, so moving cols = psum cols
            xt_list = [(0, 512), (512, 512), (1024, X1C - 1024)]
            for d in range(NCH):
                for c in range(NCH):
                    for (o, n) in xt_list:
                        psl = ps_x[:, o:o + n]
                        ksl = (slice(None), c, slice(d * P, (d + 1) * P))
                        nc.tensor.matmul(psl, k1hi[ksl], hbhi[:, c, o:o + n],
                                         start=(c == 0), stop=False)
                        nc.tensor.matmul(psl, k1hi[ksl], hblo[:, c, o:o + n],
                                         start=False, stop=False)
                        nc.tensor.matmul(psl, k1lo[ksl], hbhi[:, c, o:o + n],
                                         start=False, stop=False)
                for (o, n) in xt_list:
                    nc.tensor.matmul(
                        ps_x[:, o:o + n],
                        b1buf[:, d * P:(d + 1) * P],
                        ones1[:, o:o + n],
                        start=False, stop=True)
                nc.scalar.activation(xbuf[:, d, 0:X1C], ps_x[:, 0:X1C], AF.Copy)

            # reset scan state for module 1 (hi of par 0 and stale lo of both)
            nc.vector.memset(shl[0][:], 0.0)
            nc.vector.memset(shl[1][:, :, B:2 * B], 0.0)

            # ---- P3: module-1 scan ----
            for j in range(0, S1 - NPREC):
                cheap_step(j % 2, _x1base(j))
            for j in range(S1 - NPREC, S1):
                prec_step(j, j % 2, 1, first=(j == S1 - NPREC),
                          out_i=(j - (S1 - L) if j >= S1 - L else None))

    nc.compile()
    return nc


def _bf16_pair(x):
    import ml_dtypes
    hi = x.astype(ml_dtypes.bfloat16)
    lo = (x - hi.astype(np.float32)).astype(ml_dtypes.bfloat16)
    return hi, lo


def _host_inputs(u, kernel0, rec0, bias0, kernel1, rec1, bias1):
    u = np.asarray(u, dtype=np.float32).reshape(T, IN)
    w0hi, w0lo = _bf16_pair(0.5 * np.asarray(rec0, dtype=np.float32))
    w1hi, w1lo = _bf16_pair(0.5 * np.asarray(rec1, dtype=np.float32))
    k1hi, k1lo = _bf16_pair(0.5 * np.asarray(kernel1, dtype=np.float32))
    k0aug = np.concatenate(
        [np.asarray(kernel0, dtype=np.float32),
         np.asarray(bias0, dtype=np.float32).reshape(1, UNITS)], axis=0)
    b1row = np.asarray(bias1, dtype=np.float32).reshape(1, UNITS).copy()

    # phase-major column maps: x0 col (ph, sig) <-> t = L*sig + ph
    ph0, sg0 = np.meshgrid(np.arange(L), np.arange(-PAD0, B), indexing="ij")
    t0map = (L * sg0 + ph0).reshape(-1)          # x0 col -> core-relative time
    ph1, sg1 = np.meshgrid(np.arange(L), np.arange(-PAD1, B), indexing="ij")
    t1map = (L * sg1 + ph1).reshape(-1)

    in_maps = []
    for core in range(NCORES):
        s0 = core * SPAN
        tg = s0 + t0map                          # global times per x0 col
        u_aug = np.zeros((IN + 1, X0C), dtype=np.float32)
        ok = tg >= 0
        u_aug[:IN, ok] = u[tg[ok]].T
        u_aug[IN, ok] = 1.0
        ones1 = np.zeros((1, X1C), dtype=np.float32)
        ones1[0, (s0 + t1map) >= 0] = 1.0
        in_maps.append({
            "w0hi": w0hi, "w0lo": w0lo, "w1hi": w1hi, "w1lo": w1lo,
            "k1hi": k1hi, "k1lo": k1lo, "k0aug": k0aug,
            "b1row": b1row, "u_aug": u_aug, "ones1": ones1,
        })
    return in_maps


def _reorder(arr):
    # arr [L, P, NCH*B] -> [SPAN, UNITS]; element (i, p, c*B+s) is
    # h at (row s*L+i, col c*P+p)
    a = arr.reshape(L, P, NCH, B)
    return a.transpose(3, 0, 2, 1).reshape(SPAN, UNITS)


def kernel(u, kernel0, rec0, bias0, kernel1, rec1, bias1):
    if "nc" not in _CACHE:
        _CACHE["nc"] = _build()
    nc = _CACHE["nc"]
    in_maps = _host_inputs(u, kernel0, rec0, bias0, kernel1, rec1, bias1)
    res = run_bass_kernel_spmd(nc, in_maps, core_ids=list(range(NCORES)))
    out = np.empty((T, 2 * UNITS), dtype=np.float32)
    for c in range(NCORES):
        out[c * SPAN:(c + 1) * SPAN, :UNITS] = _reorder(res.results[c]["out0"])
        out[c * SPAN:(c + 1) * SPAN, UNITS:] = _reorder(res.results[c]["out1"])
    return out.reshape(1, T, 2 * UNITS)


# revision 30
# speedup vs baseline: 7.0068x; 7.0068x over previous
"""DeepReservoir (2-layer leaky ESN, T=8192, units=1024) on 8 trn2 cores.

Strategy: parallel-in-time with washout. Each core owns a contiguous
1024-step span, split into B=128 chunks of L=8 steps advancing in
lockstep as the free dimension of the recurrent matmuls. Chunks cold-
start from h=0 with W=32 washout steps (fading memory ~0.8/step).
Module 0 runs 2W washout so its recorded trajectory also covers module
1's washout window.

Precision is two-phase. Washout steps run pure bf16 (one matmul per
weight tile, fp32 PSUM); the last H+L steps run split-precision
(W ~ W_hi + W_lo, s ~ s_hi + s_lo, z ~ s_hi@W_hi + s_lo@W_hi +
s_hi@W_lo), giving ~1.6e-3 end-to-end error (validated against an
exact CPU model of this scheme).

All x/trajectory buffers use a phase-major column layout
col(t) = (t%L)*PW + t//L + PAD so that every per-step scan access is a
contiguous 128-column slice (time-major layouts force stride-L element
access, which runs ~4x slower on the DVE). The host permutes the input
projection columns to match; the trajectory and X1 projection share
one layout so the P2 matmul stays contiguous too.

Per step, matmuls and element-wise chains are interleaved over
unit-chunk groups (issue MM group g, then the DVE chain of group g-1)
because tile-framework semaphore thresholds follow program order:
issuing all matmuls before all DVE ops serializes the step and the
resulting PE idle gaps re-throttle the HAM clock gate to 1.2 GHz.
Trajectory records and output scaling run on the scalar engine (ACT),
which is otherwise idle between tanhs. State is tracked as s=2h so
the leaky blend is one fused DVE op; biases fold into the projection
matmuls via an augmented ones-row. Outputs are written to DRAM in the
on-chip layout and reordered on the host.
"""

import numpy as np

import concourse.bass as bass
import concourse.mybir as mybir
from concourse import bacc
from concourse.bass import ds
from concourse.tile import TileContext
from concourse.bass_utils import run_bass_kernel_spmd

# problem constants
T = 8192
UNITS = 1024
IN = 32
NCORES = 8
P = 128
NCH = UNITS // P  # 8 unit chunks

# tuning
W_WASH = 28           # washout steps
H_PREC = 8            # precise (split3) steps before the output window
B = 128               # time chunks per core (matmul free dim)
SPAN = T // NCORES    # 1024 steps per core
L = SPAN // B         # 8 steps per chunk
NPREC = H_PREC + L    # split3 steps per module (16)
S0 = 2 * W_WASH + L   # module-0 scan steps (64)
S1 = W_WASH + L       # module-1 scan steps (36)
PAD0 = -(-2 * W_WASH // L)  # x0 left pad in sigma units (7)
PAD1 = -(-W_WASH // L)      # x1/hb left pad (4)
PW0 = B + PAD0            # x0 cols per phase (135)
PW1 = B + PAD1            # x1/hb cols per phase (132)
X0C = L * PW0             # x0 columns (1080)
X1C = L * PW1             # x1 / hb columns (1056)
# DVE op groups over unit-chunks: pairs early, singles late (the last
# groups' add->tanh->blend chains gate the next step's matmuls)
GROUPS = [(0, 2), (2, 2), (4, 1), (5, 1), (6, 1), (7, 1)]

FP = mybir.dt.float32
BF = mybir.dt.bfloat16
AF = mybir.ActivationFunctionType
OP = mybir.AluOpType

_CACHE = {}


def _x0base(i):
    # contiguous col base for x0 reads at scan step i (+s for chunk s)
    return ((i - 2 * W_WASH) % L) * PW0 + (i - 2 * W_WASH) // L + PAD0


def _x1base(j):
    return ((j - W_WASH) % L) * PW1 + (j - W_WASH) // L + PAD1


def _recbase(i):
    # hb col base for the state written by mod0 step i (time 8s + i - 2W)
    return ((i - 2 * W_WASH) % L) * PW1 + (i - 2 * W_WASH) // L + PAD1


def _build():
    nc = bacc.Bacc()
    dw = {}
    for nm in ["w0hi", "w0lo", "w1hi", "w1lo", "k1hi", "k1lo"]:
        dw[nm] = nc.dram_tensor(nm, [UNITS, UNITS], BF, kind="ExternalInput")
    d_k0 = nc.dram_tensor("k0aug", [IN + 1, UNITS], FP, kind="ExternalInput")
    d_b1 = nc.dram_tensor("b1row", [1, UNITS], FP, kind="ExternalInput")
    d_u = nc.dram_tensor("u_aug", [IN + 1, X0C], FP, kind="ExternalInput")
    d_on = nc.dram_tensor("ones1", [1, X1C], FP, kind="ExternalInput")
    d_out0 = nc.dram_tensor("out0", [L, P, NCH * B], FP, kind="ExternalOutput")
    d_out1 = nc.dram_tensor("out1", [L, P, NCH * B], FP, kind="ExternalOutput")

    with TileContext(nc) as tc:
        with tc.tile_pool(name="sb", bufs=1) as pool, \
             tc.tile_pool(name="ps", bufs=1, space="PSUM") as psp:
            whi = pool.tile([P, NCH, UNITS], BF)   # W0hi, later W1hi
            wlo = pool.tile([P, NCH, UNITS], BF)   # W0lo, later W1lo
            k1hi = pool.tile([P, NCH, UNITS], BF)
            k1lo = pool.tile([P, NCH, UNITS], BF)
            k0buf = pool.tile([IN + 1, UNITS], FP)
            b1buf = pool.tile([1, UNITS], FP)
            uin = pool.tile([IN + 1, X0C], FP)
            ones1 = pool.tile([1, X1C], FP)
            xbuf = pool.tile([P, NCH, X0C], FP)    # X0x, then X1x
            hbhi = pool.tile([P, NCH, X1C], BF)    # s0 trajectory (hi)
            hblo = pool.tile([P, NCH, X1C], BF)    # s0 trajectory (lo)
            hm = [pool.tile([P, NCH, B], FP, name=f"hm{i}") for i in range(2)]
            # state hi/lo interleaved per unit-chunk: [:, c, 0:B]=hi, [:, c, B:2B]=lo
            shl = [pool.tile([P, NCH, 2 * B], BF, name=f"shl{i}") for i in range(2)]
            zg = pool.tile([P, NCH, B], FP)
            gt = pool.tile([P, NCH, B], FP)
            hout = pool.tile([P, NCH, B], FP)
            # one PSUM bank per DVE group so a group's PSUM read never
            # blocks the next group's matmul writes:
            # d0,d1->bank0  d2,d3->bank1  d4->bank2  d5->bank7  d6->bank3
            # d7->bank4; projections alternate banks {5,6,3} / {0,1,2}
            ps_s = psp.tile([P, 8, 256], FP)       # banks 0-3
            ps_s2 = psp.tile([P, 1, 512], FP)      # bank 4
            ps_x = psp.tile([P, 1024], FP)         # banks 5-6
            ps_s3 = psp.tile([P, 1, 512], FP)      # bank 7

            _PSLOT = {0: 0, 1: 1, 2: 2, 3: 3, 4: 4, 6: 6}

            def _psl(d):
                # matmul output region for unit-chunk d
                if d == 5:
                    return ps_s3[:, 0, 0:B]
                if d == 7:
                    return ps_s2[:, 0, 0:B]
                return ps_s[:, _PSLOT[d], 0:B]

            def _psg(g, gn):
                # DVE read region for group (g, gn), shaped [P, gn, B]
                if gn == 2:
                    return ps_s[:, g:g + gn, 0:B]
                if g == 5:
                    return ps_s3[:, :, 0:B]
                if g == 7:
                    return ps_s2[:, :, 0:B]
                return ps_s[:, _PSLOT[g]:_PSLOT[g] + 1, 0:B]

            # ---- preamble loads (scan-critical tensors first) ----
            nc.sync.dma_start(out=uin[:], in_=d_u[:])
            nc.sync.dma_start(out=k0buf[:], in_=d_k0[:])
            for c in range(NCH):
                nc.sync.dma_start(out=whi[:, c, :], in_=dw["w0hi"][c * P:(c + 1) * P, :])
            for c in range(NCH):
                nc.sync.dma_start(out=wlo[:, c, :], in_=dw["w0lo"][c * P:(c + 1) * P, :])
            nc.sync.dma_start(out=b1buf[:], in_=d_b1[:])
            nc.sync.dma_start(out=ones1[:], in_=d_on[:])
            for c in range(NCH):
                nc.sync.dma_start(out=k1hi[:, c, :], in_=dw["k1hi"][c * P:(c + 1) * P, :])
                nc.sync.dma_start(out=k1lo[:, c, :], in_=dw["k1lo"][c * P:(c + 1) * P, :])
            nc.vector.memset(shl[0][:], 0.0)
            nc.vector.memset(shl[1][:, :, B:2 * B], 0.0)
            nc.vector.memset(hblo[:], 0.0)

            # ---- projection helper: alternate psum buffers across d so the
            # ACT drain of one block never shares a bank with the next
            # block's matmuls ----
            def _proj_segs(d, ncols):
                n3 = ncols - 1024
                if d % 2 == 0:
                    return [(0, 512, ps_x[:, 0:512]),
                            (512, 512, ps_x[:, 512:1024]),
                            (1024, n3, ps_s[:, 6, 0:n3])]
                return [(0, 512, ps_s[:, 0:2, :]),
                        (512, 512, ps_s[:, 2:4, :]),
                        (1024, n3, ps_s[:, 4, 0:n3])]

            # ---- P0: X0x = K0aug.T @ u_aug  -> xbuf (fp32) ----
            for d in range(NCH):
                for (o, n, sl) in _proj_segs(d, X0C):
                    nc.tensor.matmul(
                        sl,
                        k0buf[:, d * P:(d + 1) * P],
                        uin[:, o:o + n],
                        start=True, stop=True)
                    nc.scalar.activation(xbuf[:, d, o:o + n], sl, AF.Copy)

            # ---- scan step (shared pipeline skeleton) ----
            # Stagger over GROUPS: emit MM(G[k]), add(G[k-1]), stt(G[k-2]),
            # post(G[k-3]); the adds run as soon as their group's matmuls
            # retire (own PSUM bank), the blend chain of the last single-
            # chunk groups finishes right behind the final matmuls.
            def run_step(mm_group, add_g, stt_g, post_g):
                ng = len(GROUPS)
                for k in range(ng + 3):
                    if k < ng:
                        mm_group(*GROUPS[k])
                    if 0 <= k - 1 < ng:
                        add_g(*GROUPS[k - 1])
                    if 0 <= k - 2 < ng:
                        stt_g(*GROUPS[k - 2])
                    if 0 <= k - 3 < ng and post_g is not None:
                        post_g(*GROUPS[k - 3])

            # ---- cheap (bf16) scan step ----
            def cheap_step(par, xb, rb=None):
                # xb: x col base (int); rb: record col base or None
                si, so = shl[par], shl[1 - par]

                def mm_group(g, gn):
                    for d in range(g, g + gn):
                        for c in range(NCH):
                            nc.tensor.matmul(
                                _psl(d), whi[:, c, d * P:(d + 1) * P],
                                si[:, c, 0:B],
                                start=(c == 0), stop=(c == NCH - 1))

                def add_g(g, gn):
                    gs = slice(g, g + gn)
                    nc.vector.tensor_tensor(
                        out=zg[:, gs, :], in0=_psg(g, gn),
                        in1=xbuf[:, gs, ds(xb, B, 1)], op=OP.add)
                    nc.scalar.activation(gt[:, gs, :], zg[:, gs, :], AF.Tanh)

                def stt_g(g, gn):
                    gs = slice(g, g + gn)
                    nc.vector.scalar_tensor_tensor(
                        out=so[:, gs, 0:B], in0=si[:, gs, 0:B], scalar=0.5,
                        in1=gt[:, gs, :], op0=OP.mult, op1=OP.add)
                    if rb is not None:
                        nc.scalar.activation(hbhi[:, gs, rb:rb + B],
                                             so[:, gs, 0:B], AF.Copy)

                run_step(mm_group, add_g, stt_g, None)

            # ---- precise (split3) scan step ----
            def prec_step(i, par, mod, first=False, out_i=None):
                hi_m, ho_m = hm[par], hm[1 - par]
                si, so = shl[par], shl[1 - par]
                xb = _x0base(i) if mod == 0 else _x1base(i)
                rb = _recbase(i) if mod == 0 else None
                if first:
                    # master state = fp32 copy of the bf16 cheap state
                    nc.vector.tensor_copy(out=hi_m[:], in_=si[:, :, 0:B])

                def mm_group(g, gn):
                    for d in range(g, g + gn):
                        psl = _psl(d)
                        for c in range(NCH):
                            wsl = (slice(None), c, slice(d * P, (d + 1) * P))
                            nc.tensor.matmul(psl, whi[wsl], si[:, c, 0:B],
                                             start=(c == 0), stop=False)
                            nc.tensor.matmul(psl, whi[wsl], si[:, c, B:2 * B],
                                             start=False, stop=False)
                            nc.tensor.matmul(psl, wlo[wsl], si[:, c, 0:B],
                                             start=False, stop=(c == NCH - 1))

                def add_g(g, gn):
                    gs = slice(g, g + gn)
                    nc.vector.tensor_tensor(
                        out=zg[:, gs, :], in0=_psg(g, gn),
                        in1=xbuf[:, gs, xb:xb + B], op=OP.add)
                    nc.scalar.activation(gt[:, gs, :], zg[:, gs, :], AF.Tanh)

                def stt_g(g, gn):
                    gs = slice(g, g + gn)
                    nc.vector.scalar_tensor_tensor(
                        out=ho_m[:, gs, :], in0=hi_m[:, gs, :], scalar=0.5,
                        in1=gt[:, gs, :], op0=OP.mult, op1=OP.add)
                    # hi half of the bf16 split (lo follows in post)
                    nc.vector.tensor_copy(out=so[:, gs, 0:B], in_=ho_m[:, gs, :])

                def post_g(g, gn):
                    gs = slice(g, g + gn)
                    nc.vector.tensor_tensor(out=so[:, gs, B:2 * B],
                                            in0=ho_m[:, gs, :], in1=so[:, gs, 0:B],
                                            op=OP.subtract)
                    if rb is not None:
                        nc.scalar.activation(hbhi[:, gs, rb:rb + B],
                                             so[:, gs, 0:B], AF.Copy)
                        nc.scalar.activation(hblo[:, gs, rb:rb + B],
                                             so[:, gs, B:2 * B], AF.Copy)

                run_step(mm_group, add_g, stt_g, post_g)
                if out_i is not None:
                    nc.scalar.activation(hout[:], ho_m[:], AF.Copy, scale=0.5)
                    dst = d_out0 if mod == 0 else d_out1
                    nc.sync.dma_start(out=dst[out_i], in_=hout[:])

            # ---- P1: module-0 scan ----
            for i in range(0, W_WASH):
                cheap_step(i % 2, _x0base(i))
            for i in range(W_WASH, S0 - NPREC):
                cheap_step(i % 2, _x0base(i), _recbase(i))
            for i in range(S0 - NPREC, S0):
                prec_step(i, i % 2, 0, first=(i == S0 - NPREC),
                          out_i=(i - (S0 - L) if i >= S0 - L else None))

            # ---- load W1 into whi/wlo (after P1's last use) ----
            for c in range(NCH):
                nc.sync.dma_start(out=whi[:, c, :], in_=dw["w1hi"][c * P:(c + 1) * P, :])
                nc.sync.dma_start(out=wlo[:, c, :], in_=dw["w1lo"][c * P:(c + 1) * P, :])

            # ---- P2: X1x = K1h.T @ s0 + b1 (ones row) -> xbuf ----
            # x1 and hb share the phase-major layout, so moving cols = psum cols
            for d in range(NCH):
                segs = _proj_segs(d, X1C)
                for c in range(NCH):
                    for (o, n, psl) in segs:
                        ksl = (slice(None), c, slice(d * P, (d + 1) * P))
                        nc.tensor.matmul(psl, k1hi[ksl], hbhi[:, c, o:o + n],
                                         start=(c == 0), stop=False)
                        nc.tensor.matmul(psl, k1hi[ksl], hblo[:, c, o:o + n],
                                         start=False, stop=False)
                        nc.tensor.matmul(psl, k1lo[ksl], hbhi[:, c, o:o + n],
                                         start=False, stop=False)
                for (o, n, psl) in segs:
                    nc.tensor.matmul(
                        psl,
                        b1buf[:, d * P:(d + 1) * P],
                        ones1[:, o:o + n],
                        start=False, stop=True)
                    nc.scalar.activation(xbuf[:, d, o:o + n], psl, AF.Copy)

            # reset scan state for module 1 (hi of par 0 and stale lo of both)
            nc.vector.memset(shl[0][:], 0.0)
            nc.vector.memset(shl[1][:, :, B:2 * B], 0.0)

            # ---- P3: module-1 scan ----
            for j in range(0, S1 - NPREC):
                cheap_step(j % 2, _x1base(j))
            for j in range(S1 - NPREC, S1):
                prec_step(j, j % 2, 1, first=(j == S1 - NPREC),
                          out_i=(j - (S1 - L) if j >= S1 - L else None))

    nc.compile()
    return nc


def _bf16_pair(x):
    import ml_dtypes
    hi = x.astype(ml_dtypes.bfloat16)
    lo = (x - hi.astype(np.float32)).astype(ml_dtypes.bfloat16)
    return hi, lo


def _host_inputs(u, kernel0, rec0, bias0, kernel1, rec1, bias1):
    u = np.asarray(u, dtype=np.float32).reshape(T, IN)
    w0hi, w0lo = _bf16_pair(0.5 * np.asarray(rec0, dtype=np.float32))
    w1hi, w1lo = _bf16_pair(0.5 * np.asarray(rec1, dtype=np.float32))
    k1hi, k1lo = _bf16_pair(0.5 * np.asarray(kernel1, dtype=np.float32))
    k0aug = np.concatenate(
        [np.asarray(kernel0, dtype=np.float32),
         np.asarray(bias0, dtype=np.float32).reshape(1, UNITS)], axis=0)
    b1row = np.asarray(bias1, dtype=np.float32).reshape(1, UNITS).copy()

    # phase-major column maps: x0 col (ph, sig) <-> t = L*sig + ph
    ph0, sg0 = np.meshgrid(np.arange(L), np.arange(-PAD0, B), indexing="ij")
    t0map = (L * sg0 + ph0).reshape(-1)          # x0 col -> core-relative time
    ph1, sg1 = np.meshgrid(np.arange(L), np.arange(-PAD1, B), indexing="ij")
    t1map = (L * sg1 + ph1).reshape(-1)

    in_maps = []
    for core in range(NCORES):
        s0 = core * SPAN
        tg = s0 + t0map                          # global times per x0 col
        u_aug = np.zeros((IN + 1, X0C), dtype=np.float32)
        ok = tg >= 0
        u_aug[:IN, ok] = u[tg[ok]].T
        u_aug[IN, ok] = 1.0
        ones1 = np.zeros((1, X1C), dtype=np.float32)
        ones1[0, (s0 + t1map) >= 0] = 1.0
        in_maps.append({
            "w0hi": w0hi, "w0lo": w0lo, "w1hi": w1hi, "w1lo": w1lo,
            "k1hi": k1hi, "k1lo": k1lo, "k0aug": k0aug,
            "b1row": b1row, "u_aug": u_aug, "ones1": ones1,
        })
    return in_maps


def _reorder(arr):
    # arr [L, P, NCH*B] -> [SPAN, UNITS]; element (i, p, c*B+s) is
    # h at (row s*L+i, col c*P+p)
    a = arr.reshape(L, P, NCH, B)
    return a.transpose(3, 0, 2, 1).reshape(SPAN, UNITS)


def kernel(u, kernel0, rec0, bias0, kernel1, rec1, bias1):
    if "nc" not in _CACHE:
        _CACHE["nc"] = _build()
    nc = _CACHE["nc"]
    in_maps = _host_inputs(u, kernel0, rec0, bias0, kernel1, rec1, bias1)
    res = run_bass_kernel_spmd(nc, in_maps, core_ids=list(range(NCORES)))
    out = np.empty((T, 2 * UNITS), dtype=np.float32)
    for c in range(NCORES):
        out[c * SPAN:(c + 1) * SPAN, :UNITS] = _reorder(res.results[c]["out0"])
        out[c * SPAN:(c + 1) * SPAN, UNITS:] = _reorder(res.results[c]["out1"])
    return out.reshape(1, T, 2 * UNITS)


# revision 32
# speedup vs baseline: 7.3393x; 1.0475x over previous
"""DeepReservoir (2-layer leaky ESN, T=8192, units=1024) on 8 trn2 cores.

Strategy: parallel-in-time with washout. Each core owns a contiguous
1024-step span, split into B=128 chunks of L=8 steps advancing in
lockstep as the free dimension of the recurrent matmuls. Chunks cold-
start from h=0 with W=32 washout steps (fading memory ~0.8/step).
Module 0 runs 2W washout so its recorded trajectory also covers module
1's washout window.

Precision is two-phase. Washout steps run pure bf16 (one matmul per
weight tile, fp32 PSUM); the last H+L steps run split-precision
(W ~ W_hi + W_lo, s ~ s_hi + s_lo, z ~ s_hi@W_hi + s_lo@W_hi +
s_hi@W_lo), giving ~1.6e-3 end-to-end error (validated against an
exact CPU model of this scheme).

All x/trajectory buffers use a phase-major column layout
col(t) = (t%L)*PW + t//L + PAD so that every per-step scan access is a
contiguous 128-column slice (time-major layouts force stride-L element
access, which runs ~4x slower on the DVE). The host permutes the input
projection columns to match; the trajectory and X1 projection share
one layout so the P2 matmul stays contiguous too.

Per step, matmuls and element-wise chains are interleaved over
unit-chunk groups (issue MM group g, then the DVE chain of group g-1)
because tile-framework semaphore thresholds follow program order:
issuing all matmuls before all DVE ops serializes the step and the
resulting PE idle gaps re-throttle the HAM clock gate to 1.2 GHz.
Trajectory records and output scaling run on the scalar engine (ACT),
which is otherwise idle between tanhs. State is tracked as s=2h so
the leaky blend is one fused DVE op; biases fold into the projection
matmuls via an augmented ones-row. Outputs are written to DRAM in the
on-chip layout and reordered on the host.
"""

import numpy as np

import concourse.bass as bass
import concourse.mybir as mybir
from concourse import bacc
from concourse.bass import ds
from concourse.tile import TileContext
from concourse.bass_utils import run_bass_kernel_spmd

# problem constants
T = 8192
UNITS = 1024
IN = 32
NCORES = 8
P = 128
NCH = UNITS // P  # 8 unit chunks

# tuning
W_WASH = 28           # washout steps
H_PREC = 4            # precise (split3) steps before the output window
B = 128               # time chunks per core (matmul free dim)
SPAN = T // NCORES    # 1024 steps per core
L = SPAN // B         # 8 steps per chunk
NPREC = H_PREC + L    # split3 steps per module (16)
S0 = 2 * W_WASH + L   # module-0 scan steps (64)
S1 = W_WASH + L       # module-1 scan steps (36)
PAD0 = -(-2 * W_WASH // L)  # x0 left pad in sigma units (7)
PAD1 = -(-W_WASH // L)      # x1/hb left pad (4)
PW0 = B + PAD0            # x0 cols per phase (135)
PW1 = B + PAD1            # x1/hb cols per phase (132)
X0C = L * PW0             # x0 columns (1080)
X1C = L * PW1             # x1 / hb columns (1056)
# DVE op groups over unit-chunks: pairs early, singles late (the last
# groups' add->tanh->blend chains gate the next step's matmuls)
GROUPS = [(0, 2), (2, 2), (4, 1), (5, 1), (6, 1), (7, 1)]

FP = mybir.dt.float32
BF = mybir.dt.bfloat16
AF = mybir.ActivationFunctionType
OP = mybir.AluOpType

_CACHE = {}


def _x0base(i):
    # contiguous col base for x0 reads at scan step i (+s for chunk s)
    return ((i - 2 * W_WASH) % L) * PW0 + (i - 2 * W_WASH) // L + PAD0


def _x1base(j):
    return ((j - W_WASH) % L) * PW1 + (j - W_WASH) // L + PAD1


def _recbase(i):
    # hb col base for the state written by mod0 step i (time 8s + i - 2W)
    return ((i - 2 * W_WASH) % L) * PW1 + (i - 2 * W_WASH) // L + PAD1


def _build():
    nc = bacc.Bacc()
    dw = {}
    for nm in ["w0hi", "w0lo", "w1hi", "w1lo", "k1hi", "k1lo"]:
        dw[nm] = nc.dram_tensor(nm, [UNITS, UNITS], BF, kind="ExternalInput")
    d_k0 = nc.dram_tensor("k0aug", [IN + 1, UNITS], FP, kind="ExternalInput")
    d_b1 = nc.dram_tensor("b1row", [1, UNITS], FP, kind="ExternalInput")
    d_u = nc.dram_tensor("u_aug", [IN + 1, X0C], FP, kind="ExternalInput")
    d_on = nc.dram_tensor("ones1", [1, X1C], FP, kind="ExternalInput")
    d_out0 = nc.dram_tensor("out0", [L, P, NCH * B], FP, kind="ExternalOutput")
    d_out1 = nc.dram_tensor("out1", [L, P, NCH * B], FP, kind="ExternalOutput")

    with TileContext(nc) as tc:
        with tc.tile_pool(name="sb", bufs=1) as pool, \
             tc.tile_pool(name="ps", bufs=1, space="PSUM") as psp:
            whi = pool.tile([P, NCH, UNITS], BF)   # W0hi, later W1hi
            wlo = pool.tile([P, NCH, UNITS], BF)   # W0lo, later W1lo
            k1hi = pool.tile([P, NCH, UNITS], BF)
            k1lo = pool.tile([P, NCH, UNITS], BF)
            k0buf = pool.tile([IN + 1, UNITS], FP)
            b1buf = pool.tile([1, UNITS], FP)
            uin = pool.tile([IN + 1, X0C], FP)
            ones1 = pool.tile([1, X1C], FP)
            xbuf = pool.tile([P, NCH, X0C], FP)    # X0x, then X1x
            hbhi = pool.tile([P, NCH, X1C], BF)    # s0 trajectory (hi)
            hblo = pool.tile([P, NCH, X1C], BF)    # s0 trajectory (lo)
            hm = [pool.tile([P, NCH, B], FP, name=f"hm{i}") for i in range(2)]
            # state hi/lo interleaved per unit-chunk: [:, c, 0:B]=hi, [:, c, B:2B]=lo
            # split into low/high chunk halves: dependency tracking is per
            # tile, so a step's first matmuls must not wait on the last
            # chunk's blend from the previous step
            shl_t = [[pool.tile([P, NCH // 2, 2 * B], BF, name=f"shl{i}{h}")
                      for h in range(2)] for i in range(2)]

            class _SView:
                # shl_t accessor: [:, c-slice, col-slice] across the halves
                def __init__(self, par):
                    self.t = shl_t[par]

                def __getitem__(self, key):
                    _, cs, col = key
                    if isinstance(cs, int):
                        return self.t[cs // 4][:, cs % 4, col]
                    lo, hi = cs.start or 0, cs.stop
                    if hi <= 4:
                        return self.t[0][:, lo:hi, col]
                    assert lo >= 4
                    return self.t[1][:, lo - 4:hi - 4, col]

            shl = [_SView(0), _SView(1)]
            zg = pool.tile([P, NCH, B], FP)
            gt = pool.tile([P, NCH, B], FP)
            hout = pool.tile([P, NCH, B], FP)
            # one PSUM bank per DVE group so a group's PSUM read never
            # blocks the next group's matmul writes:
            # d0,d1->bank0  d2,d3->bank1  d4->bank2  d5->bank7  d6->bank3
            # d7->bank4; projections alternate banks {5,6,3} / {0,1,2}
            ps_s = psp.tile([P, 8, 256], FP)       # banks 0-3
            ps_s2 = psp.tile([P, 1, 512], FP)      # bank 4
            ps_x = psp.tile([P, 1024], FP)         # banks 5-6
            ps_s3 = psp.tile([P, 1, 512], FP)      # bank 7

            _PSLOT = {0: 0, 1: 1, 2: 2, 3: 3, 4: 4, 6: 6}

            def _psl(d):
                # matmul output region for unit-chunk d
                if d == 5:
                    return ps_s3[:, 0, 0:B]
                if d == 7:
                    return ps_s2[:, 0, 0:B]
                return ps_s[:, _PSLOT[d], 0:B]

            def _psg(g, gn):
                # DVE read region for group (g, gn), shaped [P, gn, B]
                if gn == 2:
                    return ps_s[:, g:g + gn, 0:B]
                if g == 5:
                    return ps_s3[:, :, 0:B]
                if g == 7:
                    return ps_s2[:, :, 0:B]
                return ps_s[:, _PSLOT[g]:_PSLOT[g] + 1, 0:B]

            # ---- preamble loads (scan-critical tensors first) ----
            nc.sync.dma_start(out=uin[:], in_=d_u[:])
            nc.sync.dma_start(out=k0buf[:], in_=d_k0[:])
            for c in range(NCH):
                nc.sync.dma_start(out=whi[:, c, :], in_=dw["w0hi"][c * P:(c + 1) * P, :])
            for c in range(NCH):
                nc.sync.dma_start(out=wlo[:, c, :], in_=dw["w0lo"][c * P:(c + 1) * P, :])
            nc.sync.dma_start(out=b1buf[:], in_=d_b1[:])
            nc.sync.dma_start(out=ones1[:], in_=d_on[:])
            for c in range(NCH):
                nc.sync.dma_start(out=k1hi[:, c, :], in_=dw["k1hi"][c * P:(c + 1) * P, :])
                nc.sync.dma_start(out=k1lo[:, c, :], in_=dw["k1lo"][c * P:(c + 1) * P, :])
            for h in range(2):
                nc.vector.memset(shl_t[0][h][:], 0.0)
                nc.vector.memset(shl_t[1][h][:, :, B:2 * B], 0.0)
            nc.vector.memset(hblo[:], 0.0)

            # ---- projection helper: alternate psum buffers across d so the
            # ACT drain of one block never shares a bank with the next
            # block's matmuls ----
            def _proj_segs(d, ncols):
                n3 = ncols - 1024
                if d % 2 == 0:
                    return [(0, 512, ps_x[:, 0:512]),
                            (512, 512, ps_x[:, 512:1024]),
                            (1024, n3, ps_s[:, 6, 0:n3])]
                return [(0, 512, ps_s[:, 0:2, :]),
                        (512, 512, ps_s[:, 2:4, :]),
                        (1024, n3, ps_s[:, 4, 0:n3])]

            # ---- P0: X0x = K0aug.T @ u_aug  -> xbuf (fp32) ----
            for d in range(NCH):
                for (o, n, sl) in _proj_segs(d, X0C):
                    nc.tensor.matmul(
                        sl,
                        k0buf[:, d * P:(d + 1) * P],
                        uin[:, o:o + n],
                        start=True, stop=True)
                    nc.scalar.activation(xbuf[:, d, o:o + n], sl, AF.Copy)

            # ---- scan step (shared pipeline skeleton) ----
            # Stagger over GROUPS: emit MM(G[k]), add(G[k-1]), stt(G[k-2]),
            # post(G[k-3]); the adds run as soon as their group's matmuls
            # retire (own PSUM bank), the blend chain of the last single-
            # chunk groups finishes right behind the final matmuls.
            def run_step(mm_group, add_g, stt_g, post_g):
                ng = len(GROUPS)
                for k in range(ng + 3):
                    if k < ng:
                        mm_group(*GROUPS[k])
                    if 0 <= k - 1 < ng:
                        add_g(*GROUPS[k - 1])
                    if 0 <= k - 2 < ng:
                        stt_g(*GROUPS[k - 2])
                    if 0 <= k - 3 < ng and post_g is not None:
                        post_g(*GROUPS[k - 3])

            # ---- cheap (bf16) scan step ----
            def cheap_step(par, xb, rb=None):
                # xb: x col base (int); rb: record col base or None
                si, so = shl[par], shl[1 - par]

                def mm_group(g, gn):
                    for d in range(g, g + gn):
                        for c in range(NCH):
                            nc.tensor.matmul(
                                _psl(d), whi[:, c, d * P:(d + 1) * P],
                                si[:, c, 0:B],
                                start=(c == 0), stop=(c == NCH - 1))

                def add_g(g, gn):
                    gs = slice(g, g + gn)
                    nc.vector.tensor_tensor(
                        out=zg[:, gs, :], in0=_psg(g, gn),
                        in1=xbuf[:, gs, ds(xb, B, 1)], op=OP.add)
                    nc.scalar.activation(gt[:, gs, :], zg[:, gs, :], AF.Tanh)

                def stt_g(g, gn):
                    gs = slice(g, g + gn)
                    nc.vector.scalar_tensor_tensor(
                        out=so[:, gs, 0:B], in0=si[:, gs, 0:B], scalar=0.5,
                        in1=gt[:, gs, :], op0=OP.mult, op1=OP.add)
                    if rb is not None:
                        nc.scalar.activation(hbhi[:, gs, rb:rb + B],
                                             so[:, gs, 0:B], AF.Copy)

                run_step(mm_group, add_g, stt_g, None)

            # ---- precise (split3) scan step ----
            def prec_step(i, par, mod, first=False, out_i=None):
                hi_m, ho_m = hm[par], hm[1 - par]
                si, so = shl[par], shl[1 - par]
                xb = _x0base(i) if mod == 0 else _x1base(i)
                rb = _recbase(i) if mod == 0 else None
                if first:
                    # master state = fp32 copy of the bf16 cheap state
                    nc.vector.tensor_copy(out=hi_m[:, 0:4, :], in_=si[:, slice(0, 4), 0:B])
                    nc.vector.tensor_copy(out=hi_m[:, 4:8, :], in_=si[:, slice(4, 8), 0:B])

                def mm_group(g, gn):
                    for d in range(g, g + gn):
                        psl = _psl(d)
                        for c in range(NCH):
                            wsl = (slice(None), c, slice(d * P, (d + 1) * P))
                            nc.tensor.matmul(psl, whi[wsl], si[:, c, 0:B],
                                             start=(c == 0), stop=False)
                            nc.tensor.matmul(psl, whi[wsl], si[:, c, B:2 * B],
                                             start=False, stop=False)
                            nc.tensor.matmul(psl, wlo[wsl], si[:, c, 0:B],
                                             start=False, stop=(c == NCH - 1))

                def add_g(g, gn):
                    gs = slice(g, g + gn)
                    nc.vector.tensor_tensor(
                        out=zg[:, gs, :], in0=_psg(g, gn),
                        in1=xbuf[:, gs, xb:xb + B], op=OP.add)
                    nc.scalar.activation(gt[:, gs, :], zg[:, gs, :], AF.Tanh)

                def stt_g(g, gn):
                    gs = slice(g, g + gn)
                    nc.vector.scalar_tensor_tensor(
                        out=ho_m[:, gs, :], in0=hi_m[:, gs, :], scalar=0.5,
                        in1=gt[:, gs, :], op0=OP.mult, op1=OP.add)
                    # hi half of the bf16 split (lo follows in post)
                    nc.vector.tensor_copy(out=so[:, gs, 0:B], in_=ho_m[:, gs, :])

                def post_g(g, gn):
                    gs = slice(g, g + gn)
                    nc.vector.tensor_tensor(out=so[:, gs, B:2 * B],
                                            in0=ho_m[:, gs, :], in1=so[:, gs, 0:B],
                                            op=OP.subtract)
                    if rb is not None:
                        nc.scalar.activation(hbhi[:, gs, rb:rb + B],
                                             so[:, gs, 0:B], AF.Copy)
                        nc.scalar.activation(hblo[:, gs, rb:rb + B],
                                             so[:, gs, B:2 * B], AF.Copy)

                run_step(mm_group, add_g, stt_g, post_g)
                if out_i is not None:
                    nc.scalar.activation(hout[:], ho_m[:], AF.Copy, scale=0.5)
                    dst = d_out0 if mod == 0 else d_out1
                    nc.sync.dma_start(out=dst[out_i], in_=hout[:])

            # ---- P1: module-0 scan ----
            for i in range(0, W_WASH):
                cheap_step(i % 2, _x0base(i))
            for i in range(W_WASH, S0 - NPREC):
                cheap_step(i % 2, _x0base(i), _recbase(i))
            for i in range(S0 - NPREC, S0):
                prec_step(i, i % 2, 0, first=(i == S0 - NPREC),
                          out_i=(i - (S0 - L) if i >= S0 - L else None))

            # ---- load W1 into whi/wlo (after P1's last use) ----
            for c in range(NCH):
                nc.sync.dma_start(out=whi[:, c, :], in_=dw["w1hi"][c * P:(c + 1) * P, :])
                nc.sync.dma_start(out=wlo[:, c, :], in_=dw["w1lo"][c * P:(c + 1) * P, :])

            # ---- P2: X1x = K1h.T @ s0 + b1 (ones row) -> xbuf ----
            # x1 and hb share the phase-major layout, so moving cols = psum cols
            for d in range(NCH):
                segs = _proj_segs(d, X1C)
                for c in range(NCH):
                    for (o, n, psl) in segs:
                        ksl = (slice(None), c, slice(d * P, (d + 1) * P))
                        nc.tensor.matmul(psl, k1hi[ksl], hbhi[:, c, o:o + n],
                                         start=(c == 0), stop=False)
                        nc.tensor.matmul(psl, k1hi[ksl], hblo[:, c, o:o + n],
                                         start=False, stop=False)
                        nc.tensor.matmul(psl, k1lo[ksl], hbhi[:, c, o:o + n],
                                         start=False, stop=False)
                for (o, n, psl) in segs:
                    nc.tensor.matmul(
                        psl,
                        b1buf[:, d * P:(d + 1) * P],
                        ones1[:, o:o + n],
                        start=False, stop=True)
                    nc.scalar.activation(xbuf[:, d, o:o + n], psl, AF.Copy)

            # reset scan state for module 1 (hi of par 0 and stale lo of both)
            for h in range(2):
                nc.vector.memset(shl_t[0][h][:], 0.0)
                nc.vector.memset(shl_t[1][h][:, :, B:2 * B], 0.0)

            # ---- P3: module-1 scan ----
            for j in range(0, S1 - NPREC):
                cheap_step(j % 2, _x1base(j))
            for j in range(S1 - NPREC, S1):
                prec_step(j, j % 2, 1, first=(j == S1 - NPREC),
                          out_i=(j - (S1 - L) if j >= S1 - L else None))

    nc.compile()
    return nc


def _bf16_pair(x):
    import ml_dtypes
    hi = x.astype(ml_dtypes.bfloat16)
    lo = (x - hi.astype(np.float32)).astype(ml_dtypes.bfloat16)
    return hi, lo


def _host_inputs(u, kernel0, rec0, bias0, kernel1, rec1, bias1):
    u = np.asarray(u, dtype=np.float32).reshape(T, IN)
    w0hi, w0lo = _bf16_pair(0.5 * np.asarray(rec0, dtype=np.float32))
    w1hi, w1lo = _bf16_pair(0.5 * np.asarray(rec1, dtype=np.float32))
    k1hi, k1lo = _bf16_pair(0.5 * np.asarray(kernel1, dtype=np.float32))
    k0aug = np.concatenate(
        [np.asarray(kernel0, dtype=np.float32),
         np.asarray(bias0, dtype=np.float32).reshape(1, UNITS)], axis=0)
    b1row = np.asarray(bias1, dtype=np.float32).reshape(1, UNITS).copy()

    # phase-major column maps: x0 col (ph, sig) <-> t = L*sig + ph
    ph0, sg0 = np.meshgrid(np.arange(L), np.arange(-PAD0, B), indexing="ij")
    t0map = (L * sg0 + ph0).reshape(-1)          # x0 col -> core-relative time
    ph1, sg1 = np.meshgrid(np.arange(L), np.arange(-PAD1, B), indexing="ij")
    t1map = (L * sg1 + ph1).reshape(-1)

    in_maps = []
    for core in range(NCORES):
        s0 = core * SPAN
        tg = s0 + t0map                          # global times per x0 col
        u_aug = np.zeros((IN + 1, X0C), dtype=np.float32)
        ok = tg >= 0
        u_aug[:IN, ok] = u[tg[ok]].T
        u_aug[IN, ok] = 1.0
        ones1 = np.zeros((1, X1C), dtype=np.float32)
        ones1[0, (s0 + t1map) >= 0] = 1.0
        in_maps.append({
            "w0hi": w0hi, "w0lo": w0lo, "w1hi": w1hi, "w1lo": w1lo,
            "k1hi": k1hi, "k1lo": k1lo, "k0aug": k0aug,
            "b1row": b1row, "u_aug": u_aug, "ones1": ones1,
        })
    return in_maps


def _reorder(arr):
    # arr [L, P, NCH*B] -> [SPAN, UNITS]; element (i, p, c*B+s) is
    # h at (row s*L+i, col c*P+p)
    a = arr.reshape(L, P, NCH, B)
    return a.transpose(3, 0, 2, 1).reshape(SPAN, UNITS)


def kernel(u, kernel0, rec0, bias0, kernel1, rec1, bias1):
    if "nc" not in _CACHE:
        _CACHE["nc"] = _build()
    nc = _CACHE["nc"]
    in_maps = _host_inputs(u, kernel0, rec0, bias0, kernel1, rec1, bias1)
    res = run_bass_kernel_spmd(nc, in_maps, core_ids=list(range(NCORES)))
    out = np.empty((T, 2 * UNITS), dtype=np.float32)
    for c in range(NCORES):
        out[c * SPAN:(c + 1) * SPAN, :UNITS] = _reorder(res.results[c]["out0"])
        out[c * SPAN:(c + 1) * SPAN, UNITS:] = _reorder(res.results[c]["out1"])
    return out.reshape(1, T, 2 * UNITS)


# revision 33
# speedup vs baseline: 7.3615x; 1.0030x over previous
"""DeepReservoir (2-layer leaky ESN, T=8192, units=1024) on 8 trn2 cores.

Strategy: parallel-in-time with washout. Each core owns a contiguous
1024-step span, split into B=128 chunks of L=8 steps advancing in
lockstep as the free dimension of the recurrent matmuls. Chunks cold-
start from h=0 with W=32 washout steps (fading memory ~0.8/step).
Module 0 runs 2W washout so its recorded trajectory also covers module
1's washout window.

Precision is two-phase. Washout steps run pure bf16 (one matmul per
weight tile, fp32 PSUM); the last H+L steps run split-precision
(W ~ W_hi + W_lo, s ~ s_hi + s_lo, z ~ s_hi@W_hi + s_lo@W_hi +
s_hi@W_lo), giving ~1.6e-3 end-to-end error (validated against an
exact CPU model of this scheme).

All x/trajectory buffers use a phase-major column layout
col(t) = (t%L)*PW + t//L + PAD so that every per-step scan access is a
contiguous 128-column slice (time-major layouts force stride-L element
access, which runs ~4x slower on the DVE). The host permutes the input
projection columns to match; the trajectory and X1 projection share
one layout so the P2 matmul stays contiguous too.

Per step, matmuls and element-wise chains are interleaved over
unit-chunk groups (issue MM group g, then the DVE chain of group g-1)
because tile-framework semaphore thresholds follow program order:
issuing all matmuls before all DVE ops serializes the step and the
resulting PE idle gaps re-throttle the HAM clock gate to 1.2 GHz.
Trajectory records and output scaling run on the scalar engine (ACT),
which is otherwise idle between tanhs. State is tracked as s=2h so
the leaky blend is one fused DVE op; biases fold into the projection
matmuls via an augmented ones-row. Outputs are written to DRAM in the
on-chip layout and reordered on the host.
"""

import numpy as np

import concourse.bass as bass
import concourse.mybir as mybir
from concourse import bacc
from concourse.bass import ds
from concourse.tile import TileContext
from concourse.bass_utils import run_bass_kernel_spmd

# problem constants
T = 8192
UNITS = 1024
IN = 32
NCORES = 8
P = 128
NCH = UNITS // P  # 8 unit chunks

# tuning
W_WASH = 28           # washout steps
H_PREC = 4            # precise (split3) steps before the output window
B = 128               # time chunks per core (matmul free dim)
SPAN = T // NCORES    # 1024 steps per core
L = SPAN // B         # 8 steps per chunk
NPREC = H_PREC + L    # split3 steps per module (16)
S0 = 2 * W_WASH + L   # module-0 scan steps (64)
S1 = W_WASH + L       # module-1 scan steps (36)
PAD0 = -(-2 * W_WASH // L)  # x0 left pad in sigma units (7)
PAD1 = -(-W_WASH // L)      # x1/hb left pad (4)
PW0 = B + PAD0            # x0 cols per phase (135)
PW1 = B + PAD1            # x1/hb cols per phase (132)
X0C = L * PW0             # x0 columns (1080)
X1C = L * PW1             # x1 / hb columns (1056)
# DVE op groups over unit-chunks: pairs early, singles late (the last
# groups' add->tanh->blend chains gate the next step's matmuls)
GROUPS = [(0, 2), (2, 2), (4, 1), (5, 1), (6, 1), (7, 1)]

FP = mybir.dt.float32
BF = mybir.dt.bfloat16
AF = mybir.ActivationFunctionType
OP = mybir.AluOpType

_CACHE = {}


def _x0base(i):
    # contiguous col base for x0 reads at scan step i (+s for chunk s)
    return ((i - 2 * W_WASH) % L) * PW0 + (i - 2 * W_WASH) // L + PAD0


def _x1base(j):
    return ((j - W_WASH) % L) * PW1 + (j - W_WASH) // L + PAD1


def _recbase(i):
    # hb col base for the state written by mod0 step i (time 8s + i - 2W)
    return ((i - 2 * W_WASH) % L) * PW1 + (i - 2 * W_WASH) // L + PAD1


def _build():
    nc = bacc.Bacc()
    dw = {}
    for nm in ["w0hi", "w0lo", "w1hi", "w1lo", "k1hi", "k1lo"]:
        dw[nm] = nc.dram_tensor(nm, [UNITS, UNITS], BF, kind="ExternalInput")
    d_k0 = nc.dram_tensor("k0aug", [IN + 1, UNITS], FP, kind="ExternalInput")
    d_b1 = nc.dram_tensor("b1row", [1, UNITS], FP, kind="ExternalInput")
    d_u = nc.dram_tensor("u_aug", [IN + 1, X0C], FP, kind="ExternalInput")
    d_on = nc.dram_tensor("ones1", [1, X1C], FP, kind="ExternalInput")
    d_out0 = nc.dram_tensor("out0", [L, P, NCH * B], FP, kind="ExternalOutput")
    d_out1 = nc.dram_tensor("out1", [L, P, NCH * B], FP, kind="ExternalOutput")

    with TileContext(nc) as tc:
        with tc.tile_pool(name="sb", bufs=1) as pool, \
             tc.tile_pool(name="ps", bufs=1, space="PSUM") as psp:
            whi = pool.tile([P, NCH, UNITS], BF)   # W0hi, later W1hi
            wlo = pool.tile([P, NCH, UNITS], BF)   # W0lo, later W1lo
            k1hi = pool.tile([P, NCH, UNITS], BF)
            k1lo = pool.tile([P, NCH, UNITS], BF)
            k0buf = pool.tile([IN + 1, UNITS], FP)
            b1buf = pool.tile([1, UNITS], FP)
            uin = pool.tile([IN + 1, X0C], FP)
            ones1 = pool.tile([1, X1C], FP)
            xbuf = pool.tile([P, NCH, X0C], FP)    # X0x, then X1x
            hbhi = pool.tile([P, NCH, X1C], BF)    # s0 trajectory (hi)
            hblo = pool.tile([P, NCH, X1C], BF)    # s0 trajectory (lo)
            hm = [pool.tile([P, NCH, B], FP, name=f"hm{i}") for i in range(2)]
            # state hi/lo interleaved per unit-chunk: [:, c, 0:B]=hi, [:, c, B:2B]=lo
            shl = [pool.tile([P, NCH, 2 * B], BF, name=f"shl{i}") for i in range(2)]
            zg = pool.tile([P, NCH, B], FP)
            gt = pool.tile([P, NCH, B], FP)
            hout = pool.tile([P, NCH, B], FP)
            # one PSUM bank per DVE group so a group's PSUM read never
            # blocks the next group's matmul writes:
            # d0,d1->bank0  d2,d3->bank1  d4->bank2  d5->bank7  d6->bank3
            # d7->bank4; projections alternate banks {5,6,3} / {0,1,2}
            ps_s = psp.tile([P, 8, 256], FP)       # banks 0-3
            ps_s2 = psp.tile([P, 1, 512], FP)      # bank 4
            ps_x = psp.tile([P, 1024], FP)         # banks 5-6
            ps_s3 = psp.tile([P, 1, 512], FP)      # bank 7

            _PSLOT = {0: 0, 1: 1, 2: 2, 3: 3, 4: 4, 6: 6}

            def _psl(d):
                # matmul output region for unit-chunk d
                if d == 5:
                    return ps_s3[:, 0, 0:B]
                if d == 7:
                    return ps_s2[:, 0, 0:B]
                return ps_s[:, _PSLOT[d], 0:B]

            def _psg(g, gn):
                # DVE read region for group (g, gn), shaped [P, gn, B]
                if gn == 2:
                    return ps_s[:, g:g + gn, 0:B]
                if g == 5:
                    return ps_s3[:, :, 0:B]
                if g == 7:
                    return ps_s2[:, :, 0:B]
                return ps_s[:, _PSLOT[g]:_PSLOT[g] + 1, 0:B]

            # ---- preamble loads (scan-critical tensors first) ----
            nc.sync.dma_start(out=uin[:], in_=d_u[:])
            nc.sync.dma_start(out=k0buf[:], in_=d_k0[:])
            for c in range(NCH):
                nc.sync.dma_start(out=whi[:, c, :], in_=dw["w0hi"][c * P:(c + 1) * P, :])
            for c in range(NCH):
                nc.sync.dma_start(out=wlo[:, c, :], in_=dw["w0lo"][c * P:(c + 1) * P, :])
            nc.sync.dma_start(out=b1buf[:], in_=d_b1[:])
            nc.sync.dma_start(out=ones1[:], in_=d_on[:])
            for c in range(NCH):
                nc.sync.dma_start(out=k1hi[:, c, :], in_=dw["k1hi"][c * P:(c + 1) * P, :])
                nc.sync.dma_start(out=k1lo[:, c, :], in_=dw["k1lo"][c * P:(c + 1) * P, :])
            nc.vector.memset(shl[0][:], 0.0)
            nc.vector.memset(shl[1][:, :, B:2 * B], 0.0)
            nc.vector.memset(hblo[:], 0.0)

            # ---- projection helper: alternate psum buffers across d so the
            # ACT drain of one block never shares a bank with the next
            # block's matmuls ----
            def _proj_segs(d, ncols):
                n3 = ncols - 1024
                if d % 2 == 0:
                    return [(0, 512, ps_x[:, 0:512]),
                            (512, 512, ps_x[:, 512:1024]),
                            (1024, n3, ps_s[:, 6, 0:n3])]
                return [(0, 512, ps_s[:, 0:2, :]),
                        (512, 512, ps_s[:, 2:4, :]),
                        (1024, n3, ps_s[:, 4, 0:n3])]

            # ---- P0: X0x = K0aug.T @ u_aug  -> xbuf (fp32) ----
            for d in range(NCH):
                for (o, n, sl) in _proj_segs(d, X0C):
                    nc.tensor.matmul(
                        sl,
                        k0buf[:, d * P:(d + 1) * P],
                        uin[:, o:o + n],
                        start=True, stop=True)
                    nc.scalar.activation(xbuf[:, d, o:o + n], sl, AF.Copy)

            # ---- scan step (shared pipeline skeleton) ----
            # Stagger over GROUPS: emit MM(G[k]), add(G[k-1]), stt(G[k-2]),
            # post(G[k-3]); the adds run as soon as their group's matmuls
            # retire (own PSUM bank), the blend chain of the last single-
            # chunk groups finishes right behind the final matmuls.
            def run_step(mm_group, add_g, stt_g, post_g):
                ng = len(GROUPS)
                for k in range(ng + 3):
                    if k < ng:
                        mm_group(*GROUPS[k])
                    if 0 <= k - 1 < ng:
                        add_g(*GROUPS[k - 1])
                    if 0 <= k - 2 < ng:
                        stt_g(*GROUPS[k - 2])
                    if 0 <= k - 3 < ng and post_g is not None:
                        post_g(*GROUPS[k - 3])

            # ---- cheap (bf16) scan step ----
            def cheap_step(par, xb, rb=None):
                # xb: x col base (int); rb: record col base or None
                si, so = shl[par], shl[1 - par]

                def mm_group(g, gn):
                    for d in range(g, g + gn):
                        for c in range(NCH):
                            nc.tensor.matmul(
                                _psl(d), whi[:, c, d * P:(d + 1) * P],
                                si[:, c, 0:B],
                                start=(c == 0), stop=(c == NCH - 1))

                def add_g(g, gn):
                    gs = slice(g, g + gn)
                    nc.vector.tensor_tensor(
                        out=zg[:, gs, :], in0=_psg(g, gn),
                        in1=xbuf[:, gs, ds(xb, B, 1)], op=OP.add)
                    nc.scalar.activation(gt[:, gs, :], zg[:, gs, :], AF.Tanh)

                def stt_g(g, gn):
                    gs = slice(g, g + gn)
                    nc.vector.scalar_tensor_tensor(
                        out=so[:, gs, 0:B], in0=si[:, gs, 0:B], scalar=0.5,
                        in1=gt[:, gs, :], op0=OP.mult, op1=OP.add)
                    if rb is not None:
                        nc.scalar.activation(hbhi[:, gs, rb:rb + B],
                                             so[:, gs, 0:B], AF.Copy)

                run_step(mm_group, add_g, stt_g, None)

            # ---- precise (split3) scan step ----
            def prec_step(i, par, mod, first=False, out_i=None):
                hi_m, ho_m = hm[par], hm[1 - par]
                si, so = shl[par], shl[1 - par]
                xb = _x0base(i) if mod == 0 else _x1base(i)
                rb = _recbase(i) if mod == 0 else None
                if first:
                    # master state = fp32 copy of the bf16 cheap state
                    nc.vector.tensor_copy(out=hi_m[:], in_=si[:, :, 0:B])

                def mm_group(g, gn):
                    for d in range(g, g + gn):
                        psl = _psl(d)
                        for c in range(NCH):
                            wsl = (slice(None), c, slice(d * P, (d + 1) * P))
                            nc.tensor.matmul(psl, whi[wsl], si[:, c, 0:B],
                                             start=(c == 0), stop=False)
                            nc.tensor.matmul(psl, whi[wsl], si[:, c, B:2 * B],
                                             start=False, stop=False)
                            nc.tensor.matmul(psl, wlo[wsl], si[:, c, 0:B],
                                             start=False, stop=(c == NCH - 1))

                def add_g(g, gn):
                    gs = slice(g, g + gn)
                    nc.vector.tensor_tensor(
                        out=zg[:, gs, :], in0=_psg(g, gn),
                        in1=xbuf[:, gs, xb:xb + B], op=OP.add)
                    nc.scalar.activation(gt[:, gs, :], zg[:, gs, :], AF.Tanh)

                def stt_g(g, gn):
                    gs = slice(g, g + gn)
                    nc.vector.scalar_tensor_tensor(
                        out=ho_m[:, gs, :], in0=hi_m[:, gs, :], scalar=0.5,
                        in1=gt[:, gs, :], op0=OP.mult, op1=OP.add)
                    # hi half of the bf16 split (lo follows in post)
                    nc.vector.tensor_copy(out=so[:, gs, 0:B], in_=ho_m[:, gs, :])

                def post_g(g, gn):
                    gs = slice(g, g + gn)
                    nc.vector.tensor_tensor(out=so[:, gs, B:2 * B],
                                            in0=ho_m[:, gs, :], in1=so[:, gs, 0:B],
                                            op=OP.subtract)
                    if rb is not None:
                        nc.scalar.activation(hbhi[:, gs, rb:rb + B],
                                             so[:, gs, 0:B], AF.Copy)
                        nc.scalar.activation(hblo[:, gs, rb:rb + B],
                                             so[:, gs, B:2 * B], AF.Copy)

                run_step(mm_group, add_g, stt_g, post_g)
                if out_i is not None:
                    nc.scalar.activation(hout[:], ho_m[:], AF.Copy, scale=0.5)
                    dst = d_out0 if mod == 0 else d_out1
                    nc.sync.dma_start(out=dst[out_i], in_=hout[:])

            # ---- P1: module-0 scan ----
            for i in range(0, W_WASH):
                cheap_step(i % 2, _x0base(i))
            for i in range(W_WASH, S0 - NPREC):
                cheap_step(i % 2, _x0base(i), _recbase(i))
            for i in range(S0 - NPREC, S0):
                prec_step(i, i % 2, 0, first=(i == S0 - NPREC),
                          out_i=(i - (S0 - L) if i >= S0 - L else None))

            # ---- load W1 into whi/wlo (after P1's last use) ----
            for c in range(NCH):
                nc.sync.dma_start(out=whi[:, c, :], in_=dw["w1hi"][c * P:(c + 1) * P, :])
                nc.sync.dma_start(out=wlo[:, c, :], in_=dw["w1lo"][c * P:(c + 1) * P, :])

            # ---- P2: X1x = K1h.T @ s0 + b1 (ones row) -> xbuf ----
            # x1 and hb share the phase-major layout, so moving cols = psum cols
            for d in range(NCH):
                segs = _proj_segs(d, X1C)
                for c in range(NCH):
                    for (o, n, psl) in segs:
                        ksl = (slice(None), c, slice(d * P, (d + 1) * P))
                        nc.tensor.matmul(psl, k1hi[ksl], hbhi[:, c, o:o + n],
                                         start=(c == 0), stop=False)
                        nc.tensor.matmul(psl, k1hi[ksl], hblo[:, c, o:o + n],
                                         start=False, stop=False)
                        nc.tensor.matmul(psl, k1lo[ksl], hbhi[:, c, o:o + n],
                                         start=False, stop=False)
                for (o, n, psl) in segs:
                    nc.tensor.matmul(
                        psl,
                        b1buf[:, d * P:(d + 1) * P],
                        ones1[:, o:o + n],
                        start=False, stop=True)
                    nc.scalar.activation(xbuf[:, d, o:o + n], psl, AF.Copy)

            # reset scan state for module 1 (hi of par 0 and stale lo of both)
            nc.vector.memset(shl[0][:], 0.0)
            nc.vector.memset(shl[1][:, :, B:2 * B], 0.0)

            # ---- P3: module-1 scan ----
            for j in range(0, S1 - NPREC):
                cheap_step(j % 2, _x1base(j))
            for j in range(S1 - NPREC, S1):
                prec_step(j, j % 2, 1, first=(j == S1 - NPREC),
                          out_i=(j - (S1 - L) if j >= S1 - L else None))

    nc.compile()
    return nc


def _bf16_pair(x):
    import ml_dtypes
    hi = x.astype(ml_dtypes.bfloat16)
    lo = (x - hi.astype(np.float32)).astype(ml_dtypes.bfloat16)
    return hi, lo


def _host_inputs(u, kernel0, rec0, bias0, kernel1, rec1, bias1):
    u = np.asarray(u, dtype=np.float32).reshape(T, IN)
    w0hi, w0lo = _bf16_pair(0.5 * np.asarray(rec0, dtype=np.float32))
    w1hi, w1lo = _bf16_pair(0.5 * np.asarray(rec1, dtype=np.float32))
    k1hi, k1lo = _bf16_pair(0.5 * np.asarray(kernel1, dtype=np.float32))
    k0aug = np.concatenate(
        [np.asarray(kernel0, dtype=np.float32),
         np.asarray(bias0, dtype=np.float32).reshape(1, UNITS)], axis=0)
    b1row = np.asarray(bias1, dtype=np.float32).reshape(1, UNITS).copy()

    # phase-major column maps: x0 col (ph, sig) <-> t = L*sig + ph
    ph0, sg0 = np.meshgrid(np.arange(L), np.arange(-PAD0, B), indexing="ij")
    t0map = (L * sg0 + ph0).reshape(-1)          # x0 col -> core-relative time
    ph1, sg1 = np.meshgrid(np.arange(L), np.arange(-PAD1, B), indexing="ij")
    t1map = (L * sg1 + ph1).reshape(-1)

    in_maps = []
    for core in range(NCORES):
        s0 = core * SPAN
        tg = s0 + t0map                          # global times per x0 col
        u_aug = np.zeros((IN + 1, X0C), dtype=np.float32)
        ok = tg >= 0
        u_aug[:IN, ok] = u[tg[ok]].T
        u_aug[IN, ok] = 1.0
        ones1 = np.zeros((1, X1C), dtype=np.float32)
        ones1[0, (s0 + t1map) >= 0] = 1.0
        in_maps.append({
            "w0hi": w0hi, "w0lo": w0lo, "w1hi": w1hi, "w1lo": w1lo,
            "k1hi": k1hi, "k1lo": k1lo, "k0aug": k0aug,
            "b1row": b1row, "u_aug": u_aug, "ones1": ones1,
        })
    return in_maps


def _reorder(arr):
    # arr [L, P, NCH*B] -> [SPAN, UNITS]; element (i, p, c*B+s) is
    # h at (row s*L+i, col c*P+p)
    a = arr.reshape(L, P, NCH, B)
    return a.transpose(3, 0, 2, 1).reshape(SPAN, UNITS)


def kernel(u, kernel0, rec0, bias0, kernel1, rec1, bias1):
    if "nc" not in _CACHE:
        _CACHE["nc"] = _build()
    nc = _CACHE["nc"]
    in_maps = _host_inputs(u, kernel0, rec0, bias0, kernel1, rec1, bias1)
    res = run_bass_kernel_spmd(nc, in_maps, core_ids=list(range(NCORES)))
    out = np.empty((T, 2 * UNITS), dtype=np.float32)
    for c in range(NCORES):
        out[c * SPAN:(c + 1) * SPAN, :UNITS] = _reorder(res.results[c]["out0"])
        out[c * SPAN:(c + 1) * SPAN, UNITS:] = _reorder(res.results[c]["out1"])
    return out.reshape(1, T, 2 * UNITS)


# revision 35
# speedup vs baseline: 8.1732x; 1.1103x over previous
"""DeepReservoir (2-layer leaky ESN, T=8192, units=1024) on 8 trn2 cores.

Strategy: parallel-in-time with washout. Each core owns a contiguous
1024-step span, split into B=128 chunks of L=8 steps advancing in
lockstep as the free dimension of the recurrent matmuls. Chunks cold-
start from h=0 with W=28 washout steps (fading memory ~0.8/step).
Module 0 runs 2W washout so its recorded trajectory also covers module
1's washout window.

Precision is two-phase. Washout steps run pure bf16 (one matmul per
weight tile, fp32 PSUM); the last H+L=12 steps run split-precision
(W ~ W_hi + W_lo, s ~ s_hi + s_lo, z ~ s_hi@W_hi + s_lo@W_hi +
s_hi@W_lo), giving ~6.6e-3 end-to-end error (validated against an
exact CPU model of this scheme; gate is 2e-2).

All x/trajectory buffers use a phase-major column layout
col(t) = (t%L)*PW + t//L + PAD so that every per-step scan access is a
contiguous 128-column slice (time-major layouts force stride-L element
access, which runs ~4x slower on the DVE). The host permutes the input
projection columns to match; the trajectory and X1 projection share
one layout so the P2 matmul stays contiguous too.

Per step, matmuls and element-wise chains are interleaved over
unit-chunk groups (issue MM group g, then the DVE chain of group g-1)
because tile-framework semaphore thresholds follow program order:
issuing all matmuls before all DVE ops serializes the step and the
resulting PE idle gaps re-throttle the HAM clock gate to 1.2 GHz.
Trajectory records and output scaling run on the scalar engine (ACT),
which is otherwise idle between tanhs. State is tracked as s=2h so
the leaky blend is one fused DVE op; biases fold into the projection
matmuls via an augmented ones-row. Outputs are written to DRAM in the
on-chip layout and reordered on the host.
"""

import numpy as np

import concourse.bass as bass
import concourse.mybir as mybir
from concourse import bacc
from concourse.bass import ds
from concourse.tile import TileContext
from concourse.bass_utils import run_bass_kernel_spmd

# problem constants
T = 8192
UNITS = 1024
IN = 32
NCORES = 8
P = 128
NCH = UNITS // P  # 8 unit chunks

# tuning
W_WASH = 28           # mod1 washout steps / trajectory history window
W0T = 40              # mod0 washout depth (trajectory row at time -k only
                      # needs tol/c^k accuracy, so W0T << 2*W_WASH suffices)
H_PREC = 4            # precise (split3) steps before the output window
B = 128               # time chunks per core (matmul free dim)
SPAN = T // NCORES    # 1024 steps per core
L = SPAN // B         # 8 steps per chunk
NPREC = H_PREC + L    # split3 steps per module (12)
S0 = W0T + L          # module-0 scan steps (48)
S1 = W_WASH + L       # module-1 scan steps (36)
PAD0 = -(-W0T // L)         # x0 left pad in sigma units (5)
PAD1 = -(-W_WASH // L)      # x1/hb left pad (4)
PW0 = B + PAD0            # x0 cols per phase (135)
PW1 = B + PAD1            # x1/hb cols per phase (132)
X0C = L * PW0             # x0 columns (1080)
X1C = L * PW1             # x1 / hb columns (1056)
# DVE op groups over unit-chunks: pairs early, singles late (the last
# groups' add->tanh->blend chains gate the next step's matmuls)
GROUPS = [(0, 2), (2, 2), (4, 1), (5, 1), (6, 1), (7, 1)]

FP = mybir.dt.float32
BF = mybir.dt.bfloat16
AF = mybir.ActivationFunctionType
OP = mybir.AluOpType

_CACHE = {}


def _x0base(i):
    # contiguous col base for x0 reads at scan step i (+s for chunk s)
    return ((i - W0T) % L) * PW0 + (i - W0T) // L + PAD0


def _x1base(j):
    return ((j - W_WASH) % L) * PW1 + (j - W_WASH) // L + PAD1


def _recbase(i):
    # hb col base for the state written by mod0 step i (time 8s + i - W0T)
    return ((i - W0T) % L) * PW1 + (i - W0T) // L + PAD1


def _build():
    nc = bacc.Bacc()
    dw = {}
    for nm in ["w0hi", "w0lo", "w1hi", "w1lo", "k1hi", "k1lo"]:
        dw[nm] = nc.dram_tensor(nm, [UNITS, UNITS], BF, kind="ExternalInput")
    d_k0 = nc.dram_tensor("k0aug", [IN + 1, UNITS], FP, kind="ExternalInput")
    d_b1 = nc.dram_tensor("b1row", [1, UNITS], FP, kind="ExternalInput")
    d_u = nc.dram_tensor("u_aug", [IN + 1, X0C], FP, kind="ExternalInput")
    d_on = nc.dram_tensor("ones1", [1, X1C], FP, kind="ExternalInput")
    d_out0 = nc.dram_tensor("out0", [L, P, NCH * B], FP, kind="ExternalOutput")
    d_out1 = nc.dram_tensor("out1", [L, P, NCH * B], FP, kind="ExternalOutput")

    with TileContext(nc) as tc:
        with tc.tile_pool(name="sb", bufs=1) as pool, \
             tc.tile_pool(name="ps", bufs=1, space="PSUM") as psp:
            whi = pool.tile([P, NCH, UNITS], BF)   # W0hi, later W1hi
            wlo = pool.tile([P, NCH, UNITS], BF)   # W0lo, later W1lo
            k1hi = pool.tile([P, NCH, UNITS], BF)
            k1lo = pool.tile([P, NCH, UNITS], BF)
            k0buf = pool.tile([IN + 1, UNITS], FP)
            b1buf = pool.tile([1, UNITS], FP)
            uin = pool.tile([IN + 1, X0C], FP)
            ones1 = pool.tile([1, X1C], FP)
            xbuf = pool.tile([P, NCH, X0C], FP)    # X0x, then X1x
            hbhi = pool.tile([P, NCH, X1C], BF)    # s0 trajectory (hi)
            hblo = pool.tile([P, NCH, X1C], BF)    # s0 trajectory (lo)
            hm = [pool.tile([P, NCH, B], FP, name=f"hm{i}") for i in range(2)]
            # state hi/lo interleaved per unit-chunk: [:, c, 0:B]=hi, [:, c, B:2B]=lo
            shl = [pool.tile([P, NCH, 2 * B], BF, name=f"shl{i}") for i in range(2)]
            zg = pool.tile([P, NCH, B], FP)
            gt = pool.tile([P, NCH, B], FP)
            hout = pool.tile([P, NCH, B], FP)
            # one PSUM bank per DVE group so a group's PSUM read never
            # blocks the next group's matmul writes:
            # d0,d1->bank0  d2,d3->bank1  d4->bank2  d5->bank7  d6->bank3
            # d7->bank4; projections alternate banks {5,6,3} / {0,1,2}
            ps_s = psp.tile([P, 8, 256], FP)       # banks 0-3
            ps_s2 = psp.tile([P, 1, 512], FP)      # bank 4
            ps_x = psp.tile([P, 1024], FP)         # banks 5-6
            ps_s3 = psp.tile([P, 1, 512], FP)      # bank 7

            _PSLOT = {0: 0, 1: 1, 2: 2, 3: 3, 4: 4, 6: 6}

            def _psl(d):
                # matmul output region for unit-chunk d
                if d == 5:
                    return ps_s3[:, 0, 0:B]
                if d == 7:
                    return ps_s2[:, 0, 0:B]
                return ps_s[:, _PSLOT[d], 0:B]

            def _psg(g, gn):
                # DVE read region for group (g, gn), shaped [P, gn, B]
                if gn == 2:
                    return ps_s[:, g:g + gn, 0:B]
                if g == 5:
                    return ps_s3[:, :, 0:B]
                if g == 7:
                    return ps_s2[:, :, 0:B]
                return ps_s[:, _PSLOT[g]:_PSLOT[g] + 1, 0:B]

            # ---- preamble loads (scan-critical tensors first) ----
            nc.sync.dma_start(out=uin[:], in_=d_u[:])
            nc.sync.dma_start(out=k0buf[:], in_=d_k0[:])
            for c in range(NCH):
                nc.sync.dma_start(out=whi[:, c, :], in_=dw["w0hi"][c * P:(c + 1) * P, :])
            for c in range(NCH):
                nc.sync.dma_start(out=wlo[:, c, :], in_=dw["w0lo"][c * P:(c + 1) * P, :])
            nc.sync.dma_start(out=b1buf[:], in_=d_b1[:])
            nc.sync.dma_start(out=ones1[:], in_=d_on[:])
            for c in range(NCH):
                nc.sync.dma_start(out=k1hi[:, c, :], in_=dw["k1hi"][c * P:(c + 1) * P, :])
                nc.sync.dma_start(out=k1lo[:, c, :], in_=dw["k1lo"][c * P:(c + 1) * P, :])
            nc.vector.memset(shl[0][:], 0.0)
            nc.vector.memset(shl[1][:, :, B:2 * B], 0.0)
            nc.vector.memset(hblo[:], 0.0)

            # ---- projection helper: alternate psum buffers across d so the
            # ACT drain of one block never shares a bank with the next
            # block's matmuls ----
            def _proj_segs(d, ncols):
                n3 = ncols - 1024
                if d % 2 == 0:
                    return [(0, 512, ps_x[:, 0:512]),
                            (512, 512, ps_x[:, 512:1024]),
                            (1024, n3, ps_s[:, 6, 0:n3])]
                return [(0, 512, ps_s[:, 0:2, :]),
                        (512, 512, ps_s[:, 2:4, :]),
                        (1024, n3, ps_s[:, 4, 0:n3])]

            # ---- P0: X0x = K0aug.T @ u_aug  -> xbuf (fp32) ----
            for d in range(NCH):
                for (o, n, sl) in _proj_segs(d, X0C):
                    nc.tensor.matmul(
                        sl,
                        k0buf[:, d * P:(d + 1) * P],
                        uin[:, o:o + n],
                        start=True, stop=True)
                    nc.scalar.activation(xbuf[:, d, o:o + n], sl, AF.Copy)

            # ---- scan step (shared pipeline skeleton) ----
            # Stagger over GROUPS: emit MM(G[k]), add(G[k-1]), stt(G[k-2]),
            # post(G[k-3]); the adds run as soon as their group's matmuls
            # retire (own PSUM bank), the blend chain of the last single-
            # chunk groups finishes right behind the final matmuls.
            def run_step(mm_group, add_g, stt_g, post_g):
                ng = len(GROUPS)
                for k in range(ng + 3):
                    if k < ng:
                        mm_group(*GROUPS[k])
                    if 0 <= k - 1 < ng:
                        add_g(*GROUPS[k - 1])
                    if 0 <= k - 2 < ng:
                        stt_g(*GROUPS[k - 2])
                    if 0 <= k - 3 < ng and post_g is not None:
                        post_g(*GROUPS[k - 3])

            # ---- cheap (bf16) scan step ----
            def cheap_step(par, xb, rb=None):
                # xb: x col base (int); rb: record col base or None
                si, so = shl[par], shl[1 - par]

                def mm_group(g, gn):
                    for d in range(g, g + gn):
                        for c in range(NCH):
                            nc.tensor.matmul(
                                _psl(d), whi[:, c, d * P:(d + 1) * P],
                                si[:, c, 0:B],
                                start=(c == 0), stop=(c == NCH - 1))

                def add_g(g, gn):
                    gs = slice(g, g + gn)
                    nc.vector.tensor_tensor(
                        out=zg[:, gs, :], in0=_psg(g, gn),
                        in1=xbuf[:, gs, ds(xb, B, 1)], op=OP.add)
                    nc.scalar.activation(gt[:, gs, :], zg[:, gs, :], AF.Tanh)

                def stt_g(g, gn):
                    gs = slice(g, g + gn)
                    nc.vector.scalar_tensor_tensor(
                        out=so[:, gs, 0:B], in0=si[:, gs, 0:B], scalar=0.5,
                        in1=gt[:, gs, :], op0=OP.mult, op1=OP.add)
                    if rb is not None:
                        nc.scalar.activation(hbhi[:, gs, rb:rb + B],
                                             so[:, gs, 0:B], AF.Copy)

                run_step(mm_group, add_g, stt_g, None)

            # ---- precise (split3) scan step ----
            def prec_step(i, par, mod, first=False, out_i=None):
                hi_m, ho_m = hm[par], hm[1 - par]
                si, so = shl[par], shl[1 - par]
                xb = _x0base(i) if mod == 0 else _x1base(i)
                rb = _recbase(i) if mod == 0 else None
                if first:
                    # master state = fp32 copy of the bf16 cheap state
                    nc.vector.tensor_copy(out=hi_m[:], in_=si[:, :, 0:B])

                def mm_group(g, gn):
                    for d in range(g, g + gn):
                        psl = _psl(d)
                        for c in range(NCH):
                            wsl = (slice(None), c, slice(d * P, (d + 1) * P))
                            nc.tensor.matmul(psl, whi[wsl], si[:, c, 0:B],
                                             start=(c == 0), stop=False)
                            nc.tensor.matmul(psl, whi[wsl], si[:, c, B:2 * B],
                                             start=False, stop=False)
                            nc.tensor.matmul(psl, wlo[wsl], si[:, c, 0:B],
                                             start=False, stop=(c == NCH - 1))

                def add_g(g, gn):
                    gs = slice(g, g + gn)
                    nc.vector.tensor_tensor(
                        out=zg[:, gs, :], in0=_psg(g, gn),
                        in1=xbuf[:, gs, xb:xb + B], op=OP.add)
                    nc.scalar.activation(gt[:, gs, :], zg[:, gs, :], AF.Tanh)

                def stt_g(g, gn):
                    gs = slice(g, g + gn)
                    nc.vector.scalar_tensor_tensor(
                        out=ho_m[:, gs, :], in0=hi_m[:, gs, :], scalar=0.5,
                        in1=gt[:, gs, :], op0=OP.mult, op1=OP.add)
                    # hi half of the bf16 split (lo follows in post)
                    nc.vector.tensor_copy(out=so[:, gs, 0:B], in_=ho_m[:, gs, :])

                def post_g(g, gn):
                    gs = slice(g, g + gn)
                    nc.vector.tensor_tensor(out=so[:, gs, B:2 * B],
                                            in0=ho_m[:, gs, :], in1=so[:, gs, 0:B],
                                            op=OP.subtract)
                    if rb is not None:
                        nc.scalar.activation(hbhi[:, gs, rb:rb + B],
                                             so[:, gs, 0:B], AF.Copy)
                        nc.scalar.activation(hblo[:, gs, rb:rb + B],
                                             so[:, gs, B:2 * B], AF.Copy)

                run_step(mm_group, add_g, stt_g, post_g)
                if out_i is not None:
                    nc.scalar.activation(hout[:], ho_m[:], AF.Copy, scale=0.5)
                    dst = d_out0 if mod == 0 else d_out1
                    nc.sync.dma_start(out=dst[out_i], in_=hout[:])

            # ---- P1: module-0 scan ----
            for i in range(0, W0T - W_WASH):
                cheap_step(i % 2, _x0base(i))
            for i in range(W0T - W_WASH, S0 - NPREC):
                cheap_step(i % 2, _x0base(i), _recbase(i))
            for i in range(S0 - NPREC, S0):
                prec_step(i, i % 2, 0, first=(i == S0 - NPREC),
                          out_i=(i - (S0 - L) if i >= S0 - L else None))

            # ---- load W1 into whi/wlo (after P1's last use) ----
            for c in range(NCH):
                nc.sync.dma_start(out=whi[:, c, :], in_=dw["w1hi"][c * P:(c + 1) * P, :])
                nc.sync.dma_start(out=wlo[:, c, :], in_=dw["w1lo"][c * P:(c + 1) * P, :])

            # ---- P2: X1x = K1h.T @ s0 + b1 (ones row) -> xbuf ----
            # x1 and hb share the phase-major layout, so moving cols = psum cols
            for d in range(NCH):
                segs = _proj_segs(d, X1C)
                for c in range(NCH):
                    for (o, n, psl) in segs:
                        ksl = (slice(None), c, slice(d * P, (d + 1) * P))
                        nc.tensor.matmul(psl, k1hi[ksl], hbhi[:, c, o:o + n],
                                         start=(c == 0), stop=False)
                        nc.tensor.matmul(psl, k1hi[ksl], hblo[:, c, o:o + n],
                                         start=False, stop=False)
                        nc.tensor.matmul(psl, k1lo[ksl], hbhi[:, c, o:o + n],
                                         start=False, stop=False)
                for (o, n, psl) in segs:
                    nc.tensor.matmul(
                        psl,
                        b1buf[:, d * P:(d + 1) * P],
                        ones1[:, o:o + n],
                        start=False, stop=True)
                    nc.scalar.activation(xbuf[:, d, o:o + n], psl, AF.Copy)

            # reset scan state for module 1 (hi of par 0 and stale lo of both)
            nc.vector.memset(shl[0][:], 0.0)
            nc.vector.memset(shl[1][:, :, B:2 * B], 0.0)

            # ---- P3: module-1 scan ----
            for j in range(0, S1 - NPREC):
                cheap_step(j % 2, _x1base(j))
            for j in range(S1 - NPREC, S1):
                prec_step(j, j % 2, 1, first=(j == S1 - NPREC),
                          out_i=(j - (S1 - L) if j >= S1 - L else None))

    nc.compile()
    return nc


def _bf16_pair(x):
    import ml_dtypes
    hi = x.astype(ml_dtypes.bfloat16)
    lo = (x - hi.astype(np.float32)).astype(ml_dtypes.bfloat16)
    return hi, lo


def _host_inputs(u, kernel0, rec0, bias0, kernel1, rec1, bias1):
    u = np.asarray(u, dtype=np.float32).reshape(T, IN)
    w0hi, w0lo = _bf16_pair(0.5 * np.asarray(rec0, dtype=np.float32))
    w1hi, w1lo = _bf16_pair(0.5 * np.asarray(rec1, dtype=np.float32))
    k1hi, k1lo = _bf16_pair(0.5 * np.asarray(kernel1, dtype=np.float32))
    k0aug = np.concatenate(
        [np.asarray(kernel0, dtype=np.float32),
         np.asarray(bias0, dtype=np.float32).reshape(1, UNITS)], axis=0)
    b1row = np.asarray(bias1, dtype=np.float32).reshape(1, UNITS).copy()

    # phase-major column maps: x0 col (ph, sig) <-> t = L*sig + ph
    ph0, sg0 = np.meshgrid(np.arange(L), np.arange(-PAD0, B), indexing="ij")
    t0map = (L * sg0 + ph0).reshape(-1)          # x0 col -> core-relative time
    ph1, sg1 = np.meshgrid(np.arange(L), np.arange(-PAD1, B), indexing="ij")
    t1map = (L * sg1 + ph1).reshape(-1)

    in_maps = []
    for core in range(NCORES):
        s0 = core * SPAN
        tg = s0 + t0map                          # global times per x0 col
        u_aug = np.zeros((IN + 1, X0C), dtype=np.float32)
        ok = tg >= 0
        u_aug[:IN, ok] = u[tg[ok]].T
        u_aug[IN, ok] = 1.0
        ones1 = np.zeros((1, X1C), dtype=np.float32)
        ones1[0, (s0 + t1map) >= 0] = 1.0
        in_maps.append({
            "w0hi": w0hi, "w0lo": w0lo, "w1hi": w1hi, "w1lo": w1lo,
            "k1hi": k1hi, "k1lo": k1lo, "k0aug": k0aug,
            "b1row": b1row, "u_aug": u_aug, "ones1": ones1,
        })
    return in_maps


def _reorder(arr):
    # arr [L, P, NCH*B] -> [SPAN, UNITS]; element (i, p, c*B+s) is
    # h at (row s*L+i, col c*P+p)
    a = arr.reshape(L, P, NCH, B)
    return a.transpose(3, 0, 2, 1).reshape(SPAN, UNITS)


def kernel(u, kernel0, rec0, bias0, kernel1, rec1, bias1):
    if "nc" not in _CACHE:
        _CACHE["nc"] = _build()
    nc = _CACHE["nc"]
    in_maps = _host_inputs(u, kernel0, rec0, bias0, kernel1, rec1, bias1)
    res = run_bass_kernel_spmd(nc, in_maps, core_ids=list(range(NCORES)))
    out = np.empty((T, 2 * UNITS), dtype=np.float32)
    for c in range(NCORES):
        out[c * SPAN:(c + 1) * SPAN, :UNITS] = _reorder(res.results[c]["out0"])
        out[c * SPAN:(c + 1) * SPAN, UNITS:] = _reorder(res.results[c]["out1"])
    return out.reshape(1, T, 2 * UNITS)


# revision 36
# speedup vs baseline: 8.2002x; 1.0033x over previous
"""DeepReservoir (2-layer leaky ESN, T=8192, units=1024) on 8 trn2 cores.

Strategy: parallel-in-time with washout. Each core owns a contiguous
1024-step span, split into B=128 chunks of L=8 steps advancing in
lockstep as the free dimension of the recurrent matmuls. Chunks cold-
start from h=0 with W=28 washout steps (fading memory ~0.8/step).
Module 0 runs 2W washout so its recorded trajectory also covers module
1's washout window.

Precision is two-phase. Washout steps run pure bf16 (one matmul per
weight tile, fp32 PSUM); the last H+L=12 steps run split-precision
(W ~ W_hi + W_lo, s ~ s_hi + s_lo, z ~ s_hi@W_hi + s_lo@W_hi +
s_hi@W_lo), giving ~6.6e-3 end-to-end error (validated against an
exact CPU model of this scheme; gate is 2e-2).

All x/trajectory buffers use a phase-major column layout
col(t) = (t%L)*PW + t//L + PAD so that every per-step scan access is a
contiguous 128-column slice (time-major layouts force stride-L element
access, which runs ~4x slower on the DVE). The host permutes the input
projection columns to match; the trajectory and X1 projection share
one layout so the P2 matmul stays contiguous too.

Per step, matmuls and element-wise chains are interleaved over
unit-chunk groups (issue MM group g, then the DVE chain of group g-1)
because tile-framework semaphore thresholds follow program order:
issuing all matmuls before all DVE ops serializes the step and the
resulting PE idle gaps re-throttle the HAM clock gate to 1.2 GHz.
Trajectory records and output scaling run on the scalar engine (ACT),
which is otherwise idle between tanhs. State is tracked as s=2h so
the leaky blend is one fused DVE op; biases fold into the projection
matmuls via an augmented ones-row. Outputs are written to DRAM in the
on-chip layout and reordered on the host.
"""

import numpy as np

import concourse.bass as bass
import concourse.mybir as mybir
from concourse import bacc
from concourse.bass import ds
from concourse.tile import TileContext
from concourse.bass_utils import run_bass_kernel_spmd

# problem constants
T = 8192
UNITS = 1024
IN = 32
NCORES = 8
P = 128
NCH = UNITS // P  # 8 unit chunks

# tuning
W_WASH = 28           # mod1 washout steps / trajectory history window
W0T = 40              # mod0 washout depth (trajectory row at time -k only
                      # needs tol/c^k accuracy, so W0T << 2*W_WASH suffices)
H_PREC = 4            # precise (split3) steps before the output window
B = 128               # time chunks per core (matmul free dim)
SPAN = T // NCORES    # 1024 steps per core
L = SPAN // B         # 8 steps per chunk
NPREC = H_PREC + L    # split3 steps per module (12)
S0 = W0T + L          # module-0 scan steps (48)
S1 = W_WASH + L       # module-1 scan steps (36)
PAD0 = -(-W0T // L)         # x0 left pad in sigma units (5)
PAD1 = -(-W_WASH // L)      # x1/hb left pad (4)
PW0 = B + PAD0            # x0 cols per phase (135)
PW1 = B + PAD1            # x1/hb cols per phase (132)
X0C = L * PW0             # x0 columns (1080)
X1C = L * PW1             # x1 / hb columns (1056)
# DVE op groups over unit-chunks: pairs early, singles late (the last
# groups' add->tanh->blend chains gate the next step's matmuls)
GROUPS = [(0, 2), (2, 2), (4, 1), (5, 1), (6, 1), (7, 1)]

FP = mybir.dt.float32
BF = mybir.dt.bfloat16
AF = mybir.ActivationFunctionType
OP = mybir.AluOpType

_CACHE = {}


def _x0base(i):
    # contiguous col base for x0 reads at scan step i (+s for chunk s)
    return ((i - W0T) % L) * PW0 + (i - W0T) // L + PAD0


def _x1base(j):
    return ((j - W_WASH) % L) * PW1 + (j - W_WASH) // L + PAD1


def _recbase(i):
    # hb col base for the state written by mod0 step i (time 8s + i - W0T)
    return ((i - W0T) % L) * PW1 + (i - W0T) // L + PAD1


def _build():
    nc = bacc.Bacc()
    dw = {}
    for nm in ["w0hi", "w0lo", "w1hi", "w1lo", "k1hi", "k1lo"]:
        dw[nm] = nc.dram_tensor(nm, [UNITS, UNITS], BF, kind="ExternalInput")
    d_k0 = nc.dram_tensor("k0aug", [IN + 1, UNITS], FP, kind="ExternalInput")
    d_b1 = nc.dram_tensor("b1row", [1, UNITS], FP, kind="ExternalInput")
    d_u = nc.dram_tensor("u_aug", [IN + 1, X0C], FP, kind="ExternalInput")
    d_on = nc.dram_tensor("ones1", [1, X1C], FP, kind="ExternalInput")
    d_out0 = nc.dram_tensor("out0", [L, P, NCH * B], FP, kind="ExternalOutput")
    d_out1 = nc.dram_tensor("out1", [L, P, NCH * B], FP, kind="ExternalOutput")

    with TileContext(nc) as tc:
        with tc.tile_pool(name="sb", bufs=1) as pool, \
             tc.tile_pool(name="ps", bufs=1, space="PSUM") as psp:
            whi = pool.tile([P, NCH, UNITS], BF)   # W0hi, later W1hi
            wlo = pool.tile([P, NCH, UNITS], BF)   # W0lo, later W1lo
            k1hi = pool.tile([P, NCH, UNITS], BF)
            k1lo = pool.tile([P, NCH, UNITS], BF)
            k0buf = pool.tile([IN + 1, UNITS], FP)
            b1buf = pool.tile([1, UNITS], FP)
            uin = pool.tile([IN + 1, X0C], FP)
            ones1 = pool.tile([1, X1C], FP)
            xbuf = pool.tile([P, NCH, X0C], FP)    # X0x, then X1x
            hbhi = pool.tile([P, NCH, X1C], BF)    # s0 trajectory (hi)
            hblo = pool.tile([P, NCH, X1C], BF)    # s0 trajectory (lo)
            hm = [pool.tile([P, NCH, B], FP, name=f"hm{i}") for i in range(2)]
            # state hi/lo interleaved per unit-chunk: [:, c, 0:B]=hi, [:, c, B:2B]=lo
            shl = [pool.tile([P, NCH, 2 * B], BF, name=f"shl{i}") for i in range(2)]
            zg = pool.tile([P, NCH, B], FP)
            gt = pool.tile([P, NCH, B], FP)
            hout = pool.tile([P, NCH, B], FP)
            # one PSUM bank per DVE group so a group's PSUM read never
            # blocks the next group's matmul writes:
            # d0,d1->bank0  d2,d3->bank1  d4->bank2  d5->bank7  d6->bank3
            # d7->bank4; projections alternate banks {5,6,3} / {0,1,2}
            ps_s = psp.tile([P, 8, 256], FP)       # banks 0-3
            ps_s2 = psp.tile([P, 1, 512], FP)      # bank 4
            ps_x = psp.tile([P, 1024], FP)         # banks 5-6
            ps_s3 = psp.tile([P, 1, 512], FP)      # bank 7

            _PSLOT = {0: 0, 1: 1, 2: 2, 3: 3, 4: 4, 6: 6}

            def _psl(d):
                # matmul output region for unit-chunk d
                if d == 5:
                    return ps_s3[:, 0, 0:B]
                if d == 7:
                    return ps_s2[:, 0, 0:B]
                return ps_s[:, _PSLOT[d], 0:B]

            def _psg(g, gn):
                # DVE read region for group (g, gn), shaped [P, gn, B]
                if gn == 2:
                    return ps_s[:, g:g + gn, 0:B]
                if g == 5:
                    return ps_s3[:, :, 0:B]
                if g == 7:
                    return ps_s2[:, :, 0:B]
                return ps_s[:, _PSLOT[g]:_PSLOT[g] + 1, 0:B]

            # ---- preamble loads (scan-critical tensors first) ----
            nc.sync.dma_start(out=uin[:], in_=d_u[:])
            nc.sync.dma_start(out=k0buf[:], in_=d_k0[:])
            for c in range(NCH):
                nc.sync.dma_start(out=whi[:, c, :], in_=dw["w0hi"][c * P:(c + 1) * P, :])
            for c in range(NCH):
                nc.sync.dma_start(out=wlo[:, c, :], in_=dw["w0lo"][c * P:(c + 1) * P, :])
            nc.sync.dma_start(out=b1buf[:], in_=d_b1[:])
            nc.sync.dma_start(out=ones1[:], in_=d_on[:])
            for c in range(NCH):
                nc.sync.dma_start(out=k1hi[:, c, :], in_=dw["k1hi"][c * P:(c + 1) * P, :])
                nc.sync.dma_start(out=k1lo[:, c, :], in_=dw["k1lo"][c * P:(c + 1) * P, :])
            nc.vector.memset(shl[0][:], 0.0)
            nc.vector.memset(shl[1][:, :, B:2 * B], 0.0)
            nc.vector.memset(hblo[:], 0.0)

            # ---- projection helper: alternate psum buffers across d so the
            # ACT drain of one block never shares a bank with the next
            # block's matmuls ----
            def _proj_segs(d, ncols):
                n3 = ncols - 1024
                if d % 2 == 0:
                    return [(0, 512, ps_x[:, 0:512]),
                            (512, 512, ps_x[:, 512:1024]),
                            (1024, n3, ps_s[:, 6, 0:n3])]
                return [(0, 512, ps_s[:, 0:2, :]),
                        (512, 512, ps_s[:, 2:4, :]),
                        (1024, n3, ps_s[:, 4, 0:n3])]

            # ---- P0: X0x = K0aug.T @ u_aug  -> xbuf (fp32) ----
            for d in range(NCH):
                for (o, n, sl) in _proj_segs(d, X0C):
                    nc.tensor.matmul(
                        sl,
                        k0buf[:, d * P:(d + 1) * P],
                        uin[:, o:o + n],
                        start=True, stop=True)
                    nc.scalar.activation(xbuf[:, d, o:o + n], sl, AF.Copy)

            # ---- scan step (shared pipeline skeleton) ----
            # Stagger over GROUPS: emit MM(G[k]), add(G[k-1]), stt(G[k-2]),
            # post(G[k-3]); the adds run as soon as their group's matmuls
            # retire (own PSUM bank), the blend chain of the last single-
            # chunk groups finishes right behind the final matmuls.
            def run_step(mm_group, add_g, stt_g, post_g):
                ng = len(GROUPS)
                for k in range(ng + 3):
                    if k < ng:
                        mm_group(*GROUPS[k])
                    if 0 <= k - 1 < ng:
                        add_g(*GROUPS[k - 1])
                    if 0 <= k - 2 < ng:
                        stt_g(*GROUPS[k - 2])
                    if 0 <= k - 3 < ng and post_g is not None:
                        post_g(*GROUPS[k - 3])

            # ---- cheap (bf16) scan step ----
            def cheap_step(par, xb, rb=None):
                # xb: x col base (int); rb: record col base or None
                si, so = shl[par], shl[1 - par]

                def mm_group(g, gn):
                    for d in range(g, g + gn):
                        for c in range(NCH):
                            nc.tensor.matmul(
                                _psl(d), whi[:, c, d * P:(d + 1) * P],
                                si[:, c, 0:B],
                                start=(c == 0), stop=(c == NCH - 1))

                def add_g(g, gn):
                    gs = slice(g, g + gn)
                    nc.vector.tensor_tensor(
                        out=zg[:, gs, :], in0=_psg(g, gn),
                        in1=xbuf[:, gs, ds(xb, B, 1)], op=OP.add)
                    nc.scalar.activation(gt[:, gs, :], zg[:, gs, :], AF.Tanh)

                def stt_g(g, gn):
                    gs = slice(g, g + gn)
                    nc.vector.scalar_tensor_tensor(
                        out=so[:, gs, 0:B], in0=si[:, gs, 0:B], scalar=0.5,
                        in1=gt[:, gs, :], op0=OP.mult, op1=OP.add)

                run_step(mm_group, add_g, stt_g, None)
                # records go last: they aren't read until P2, and issuing
                # them inside the pipeline delays the critical tanh chain
                # in the ACT FIFO
                if rb is not None:
                    nc.scalar.activation(hbhi[:, :, rb:rb + B],
                                         so[:, :, 0:B], AF.Copy)

            # ---- precise (split3) scan step ----
            def prec_step(i, par, mod, first=False, out_i=None):
                hi_m, ho_m = hm[par], hm[1 - par]
                si, so = shl[par], shl[1 - par]
                xb = _x0base(i) if mod == 0 else _x1base(i)
                rb = _recbase(i) if mod == 0 else None
                if first:
                    # master state = fp32 copy of the bf16 cheap state
                    nc.vector.tensor_copy(out=hi_m[:], in_=si[:, :, 0:B])

                def mm_group(g, gn):
                    for d in range(g, g + gn):
                        psl = _psl(d)
                        for c in range(NCH):
                            wsl = (slice(None), c, slice(d * P, (d + 1) * P))
                            nc.tensor.matmul(psl, whi[wsl], si[:, c, 0:B],
                                             start=(c == 0), stop=False)
                            nc.tensor.matmul(psl, whi[wsl], si[:, c, B:2 * B],
                                             start=False, stop=False)
                            nc.tensor.matmul(psl, wlo[wsl], si[:, c, 0:B],
                                             start=False, stop=(c == NCH - 1))

                def add_g(g, gn):
                    gs = slice(g, g + gn)
                    nc.vector.tensor_tensor(
                        out=zg[:, gs, :], in0=_psg(g, gn),
                        in1=xbuf[:, gs, xb:xb + B], op=OP.add)
                    nc.scalar.activation(gt[:, gs, :], zg[:, gs, :], AF.Tanh)

                def stt_g(g, gn):
                    gs = slice(g, g + gn)
                    nc.vector.scalar_tensor_tensor(
                        out=ho_m[:, gs, :], in0=hi_m[:, gs, :], scalar=0.5,
                        in1=gt[:, gs, :], op0=OP.mult, op1=OP.add)
                    # hi half of the bf16 split (lo follows in post)
                    nc.vector.tensor_copy(out=so[:, gs, 0:B], in_=ho_m[:, gs, :])

                def post_g(g, gn):
                    gs = slice(g, g + gn)
                    nc.vector.tensor_tensor(out=so[:, gs, B:2 * B],
                                            in0=ho_m[:, gs, :], in1=so[:, gs, 0:B],
                                            op=OP.subtract)

                run_step(mm_group, add_g, stt_g, post_g)
                # records last (not read until P2); output scale on the DVE
                # whose queue drains before the next step's first add anyway
                if rb is not None:
                    nc.scalar.activation(hbhi[:, :, rb:rb + B],
                                         so[:, :, 0:B], AF.Copy)
                    nc.scalar.activation(hblo[:, :, rb:rb + B],
                                         so[:, :, B:2 * B], AF.Copy)
                if out_i is not None:
                    nc.vector.tensor_scalar_mul(hout[:], ho_m[:], 0.5)
                    dst = d_out0 if mod == 0 else d_out1
                    nc.sync.dma_start(out=dst[out_i], in_=hout[:])

            # ---- P1: module-0 scan ----
            for i in range(0, W0T - W_WASH):
                cheap_step(i % 2, _x0base(i))
            for i in range(W0T - W_WASH, S0 - NPREC):
                cheap_step(i % 2, _x0base(i), _recbase(i))
            for i in range(S0 - NPREC, S0):
                prec_step(i, i % 2, 0, first=(i == S0 - NPREC),
                          out_i=(i - (S0 - L) if i >= S0 - L else None))

            # ---- load W1 into whi/wlo (after P1's last use) ----
            for c in range(NCH):
                nc.sync.dma_start(out=whi[:, c, :], in_=dw["w1hi"][c * P:(c + 1) * P, :])
                nc.sync.dma_start(out=wlo[:, c, :], in_=dw["w1lo"][c * P:(c + 1) * P, :])

            # ---- P2: X1x = K1h.T @ s0 + b1 (ones row) -> xbuf ----
            # x1 and hb share the phase-major layout, so moving cols = psum cols
            for d in range(NCH):
                segs = _proj_segs(d, X1C)
                for c in range(NCH):
                    for (o, n, psl) in segs:
                        ksl = (slice(None), c, slice(d * P, (d + 1) * P))
                        nc.tensor.matmul(psl, k1hi[ksl], hbhi[:, c, o:o + n],
                                         start=(c == 0), stop=False)
                        nc.tensor.matmul(psl, k1hi[ksl], hblo[:, c, o:o + n],
                                         start=False, stop=False)
                        nc.tensor.matmul(psl, k1lo[ksl], hbhi[:, c, o:o + n],
                                         start=False, stop=False)
                for (o, n, psl) in segs:
                    nc.tensor.matmul(
                        psl,
                        b1buf[:, d * P:(d + 1) * P],
                        ones1[:, o:o + n],
                        start=False, stop=True)
                    nc.scalar.activation(xbuf[:, d, o:o + n], psl, AF.Copy)

            # reset scan state for module 1 (hi of par 0 and stale lo of both)
            nc.vector.memset(shl[0][:], 0.0)
            nc.vector.memset(shl[1][:, :, B:2 * B], 0.0)

            # ---- P3: module-1 scan ----
            for j in range(0, S1 - NPREC):
                cheap_step(j % 2, _x1base(j))
            for j in range(S1 - NPREC, S1):
                prec_step(j, j % 2, 1, first=(j == S1 - NPREC),
                          out_i=(j - (S1 - L) if j >= S1 - L else None))

    nc.compile()
    return nc


def _bf16_pair(x):
    import ml_dtypes
    hi = x.astype(ml_dtypes.bfloat16)
    lo = (x - hi.astype(np.float32)).astype(ml_dtypes.bfloat16)
    return hi, lo


def _host_inputs(u, kernel0, rec0, bias0, kernel1, rec1, bias1):
    u = np.asarray(u, dtype=np.float32).reshape(T, IN)
    w0hi, w0lo = _bf16_pair(0.5 * np.asarray(rec0, dtype=np.float32))
    w1hi, w1lo = _bf16_pair(0.5 * np.asarray(rec1, dtype=np.float32))
    k1hi, k1lo = _bf16_pair(0.5 * np.asarray(kernel1, dtype=np.float32))
    k0aug = np.concatenate(
        [np.asarray(kernel0, dtype=np.float32),
         np.asarray(bias0, dtype=np.float32).reshape(1, UNITS)], axis=0)
    b1row = np.asarray(bias1, dtype=np.float32).reshape(1, UNITS).copy()

    # phase-major column maps: x0 col (ph, sig) <-> t = L*sig + ph
    ph0, sg0 = np.meshgrid(np.arange(L), np.arange(-PAD0, B), indexing="ij")
    t0map = (L * sg0 + ph0).reshape(-1)          # x0 col -> core-relative time
    ph1, sg1 = np.meshgrid(np.arange(L), np.arange(-PAD1, B), indexing="ij")
    t1map = (L * sg1 + ph1).reshape(-1)

    in_maps = []
    for core in range(NCORES):
        s0 = core * SPAN
        tg = s0 + t0map                          # global times per x0 col
        u_aug = np.zeros((IN + 1, X0C), dtype=np.float32)
        ok = tg >= 0
        u_aug[:IN, ok] = u[tg[ok]].T
        u_aug[IN, ok] = 1.0
        ones1 = np.zeros((1, X1C), dtype=np.float32)
        ones1[0, (s0 + t1map) >= 0] = 1.0
        in_maps.append({
            "w0hi": w0hi, "w0lo": w0lo, "w1hi": w1hi, "w1lo": w1lo,
            "k1hi": k1hi, "k1lo": k1lo, "k0aug": k0aug,
            "b1row": b1row, "u_aug": u_aug, "ones1": ones1,
        })
    return in_maps


def _reorder(arr):
    # arr [L, P, NCH*B] -> [SPAN, UNITS]; element (i, p, c*B+s) is
    # h at (row s*L+i, col c*P+p)
    a = arr.reshape(L, P, NCH, B)
    return a.transpose(3, 0, 2, 1).reshape(SPAN, UNITS)


def kernel(u, kernel0, rec0, bias0, kernel1, rec1, bias1):
    if "nc" not in _CACHE:
        _CACHE["nc"] = _build()
    nc = _CACHE["nc"]
    in_maps = _host_inputs(u, kernel0, rec0, bias0, kernel1, rec1, bias1)
    res = run_bass_kernel_spmd(nc, in_maps, core_ids=list(range(NCORES)))
    out = np.empty((T, 2 * UNITS), dtype=np.float32)
    for c in range(NCORES):
        out[c * SPAN:(c + 1) * SPAN, :UNITS] = _reorder(res.results[c]["out0"])
        out[c * SPAN:(c + 1) * SPAN, UNITS:] = _reorder(res.results[c]["out1"])
    return out.reshape(1, T, 2 * UNITS)


# revision 37
# speedup vs baseline: 8.4405x; 1.0293x over previous
"""DeepReservoir (2-layer leaky ESN, T=8192, units=1024) on 8 trn2 cores.

Strategy: parallel-in-time with washout. Each core owns a contiguous
1024-step span, split into B=128 chunks of L=8 steps advancing in
lockstep as the free dimension of the recurrent matmuls. Chunks cold-
start from h=0 with W=28 washout steps (fading memory ~0.8/step).
Module 0 runs 2W washout so its recorded trajectory also covers module
1's washout window.

Precision is two-phase. Washout steps run pure bf16 (one matmul per
weight tile, fp32 PSUM); the last H+L=12 steps run split-precision
(W ~ W_hi + W_lo, s ~ s_hi + s_lo, z ~ s_hi@W_hi + s_lo@W_hi +
s_hi@W_lo), giving ~6.6e-3 end-to-end error (validated against an
exact CPU model of this scheme; gate is 2e-2).

All x/trajectory buffers use a phase-major column layout
col(t) = (t%L)*PW + t//L + PAD so that every per-step scan access is a
contiguous 128-column slice (time-major layouts force stride-L element
access, which runs ~4x slower on the DVE). The host permutes the input
projection columns to match; the trajectory and X1 projection share
one layout so the P2 matmul stays contiguous too.

Per step, matmuls and element-wise chains are interleaved over
unit-chunk groups (issue MM group g, then the DVE chain of group g-1)
because tile-framework semaphore thresholds follow program order:
issuing all matmuls before all DVE ops serializes the step and the
resulting PE idle gaps re-throttle the HAM clock gate to 1.2 GHz.
Trajectory records and output scaling run on the scalar engine (ACT),
which is otherwise idle between tanhs. State is tracked as s=2h so
the leaky blend is one fused DVE op; biases fold into the projection
matmuls via an augmented ones-row. Outputs are written to DRAM in the
on-chip layout and reordered on the host.
"""

import numpy as np

import concourse.bass as bass
import concourse.mybir as mybir
from concourse import bacc
from concourse.bass import ds
from concourse.tile import TileContext
from concourse.bass_utils import run_bass_kernel_spmd

# problem constants
T = 8192
UNITS = 1024
IN = 32
NCORES = 8
P = 128
NCH = UNITS // P  # 8 unit chunks

# tuning
W_WASH = 28           # mod1 washout steps / trajectory history window
W0T = 36              # mod0 washout depth (trajectory row at time -k only
                      # needs tol/c^k accuracy, so W0T << 2*W_WASH suffices)
H_PREC = 4            # precise (split3) steps before the output window
B = 128               # time chunks per core (matmul free dim)
SPAN = T // NCORES    # 1024 steps per core
L = SPAN // B         # 8 steps per chunk
NPREC = H_PREC + L    # split3 steps per module (12)
S0 = W0T + L          # module-0 scan steps (48)
S1 = W_WASH + L       # module-1 scan steps (36)
PAD0 = -(-W0T // L)         # x0 left pad in sigma units (5)
PAD1 = -(-W_WASH // L)      # x1/hb left pad (4)
PW0 = B + PAD0            # x0 cols per phase (135)
PW1 = B + PAD1            # x1/hb cols per phase (132)
X0C = L * PW0             # x0 columns (1080)
X1C = L * PW1             # x1 / hb columns (1056)
# DVE op groups over unit-chunks: pairs early, singles late (the last
# groups' add->tanh->blend chains gate the next step's matmuls)
GROUPS = [(0, 2), (2, 2), (4, 1), (5, 1), (6, 1), (7, 1)]

FP = mybir.dt.float32
BF = mybir.dt.bfloat16
AF = mybir.ActivationFunctionType
OP = mybir.AluOpType

_CACHE = {}


def _x0base(i):
    # contiguous col base for x0 reads at scan step i (+s for chunk s)
    return ((i - W0T) % L) * PW0 + (i - W0T) // L + PAD0


def _x1base(j):
    return ((j - W_WASH) % L) * PW1 + (j - W_WASH) // L + PAD1


def _recbase(i):
    # hb col base for the state written by mod0 step i (time 8s + i - W0T)
    return ((i - W0T) % L) * PW1 + (i - W0T) // L + PAD1


def _build():
    nc = bacc.Bacc()
    dw = {}
    for nm in ["w0hi", "w0lo", "w1hi", "w1lo", "k1hi", "k1lo"]:
        dw[nm] = nc.dram_tensor(nm, [UNITS, UNITS], BF, kind="ExternalInput")
    d_k0 = nc.dram_tensor("k0aug", [IN + 1, UNITS], FP, kind="ExternalInput")
    d_b1 = nc.dram_tensor("b1row", [1, UNITS], FP, kind="ExternalInput")
    d_u = nc.dram_tensor("u_aug", [IN + 1, X0C], FP, kind="ExternalInput")
    d_on = nc.dram_tensor("ones1", [1, X1C], FP, kind="ExternalInput")
    d_out0 = nc.dram_tensor("out0", [L, P, NCH * B], FP, kind="ExternalOutput")
    d_out1 = nc.dram_tensor("out1", [L, P, NCH * B], FP, kind="ExternalOutput")

    with TileContext(nc) as tc:
        with tc.tile_pool(name="sb", bufs=1) as pool, \
             tc.tile_pool(name="ps", bufs=1, space="PSUM") as psp:
            whi = pool.tile([P, NCH, UNITS], BF)   # W0hi, later W1hi
            wlo = pool.tile([P, NCH, UNITS], BF)   # W0lo, later W1lo
            k1hi = pool.tile([P, NCH, UNITS], BF)
            k1lo = pool.tile([P, NCH, UNITS], BF)
            k0buf = pool.tile([IN + 1, UNITS], FP)
            b1buf = pool.tile([1, UNITS], FP)
            uin = pool.tile([IN + 1, X0C], FP)
            ones1 = pool.tile([1, X1C], FP)
            xbuf = pool.tile([P, NCH, X0C], FP)    # X0x, then X1x
            hbhi = pool.tile([P, NCH, X1C], BF)    # s0 trajectory (hi)
            hblo = pool.tile([P, NCH, X1C], BF)    # s0 trajectory (lo)
            hm = [pool.tile([P, NCH, B], FP, name=f"hm{i}") for i in range(2)]
            # state hi/lo interleaved per unit-chunk: [:, c, 0:B]=hi, [:, c, B:2B]=lo
            shl = [pool.tile([P, NCH, 2 * B], BF, name=f"shl{i}") for i in range(2)]
            zg = pool.tile([P, NCH, B], FP)
            gt = pool.tile([P, NCH, B], FP)
            hout = pool.tile([P, NCH, B], FP)
            # one PSUM bank per DVE group so a group's PSUM read never
            # blocks the next group's matmul writes:
            # d0,d1->bank0  d2,d3->bank1  d4->bank2  d5->bank7  d6->bank3
            # d7->bank4; projections alternate banks {5,6,3} / {0,1,2}
            ps_s = psp.tile([P, 8, 256], FP)       # banks 0-3
            ps_s2 = psp.tile([P, 1, 512], FP)      # bank 4
            ps_x = psp.tile([P, 1024], FP)         # banks 5-6
            ps_s3 = psp.tile([P, 1, 512], FP)      # bank 7

            _PSLOT = {0: 0, 1: 1, 2: 2, 3: 3, 4: 4, 6: 6}

            def _psl(d):
                # matmul output region for unit-chunk d
                if d == 5:
                    return ps_s3[:, 0, 0:B]
                if d == 7:
                    return ps_s2[:, 0, 0:B]
                return ps_s[:, _PSLOT[d], 0:B]

            def _psg(g, gn):
                # DVE read region for group (g, gn), shaped [P, gn, B]
                if gn == 2:
                    return ps_s[:, g:g + gn, 0:B]
                if g == 5:
                    return ps_s3[:, :, 0:B]
                if g == 7:
                    return ps_s2[:, :, 0:B]
                return ps_s[:, _PSLOT[g]:_PSLOT[g] + 1, 0:B]

            # ---- preamble loads (scan-critical tensors first) ----
            nc.sync.dma_start(out=uin[:], in_=d_u[:])
            nc.sync.dma_start(out=k0buf[:], in_=d_k0[:])
            for c in range(NCH):
                nc.sync.dma_start(out=whi[:, c, :], in_=dw["w0hi"][c * P:(c + 1) * P, :])
            for c in range(NCH):
                nc.sync.dma_start(out=wlo[:, c, :], in_=dw["w0lo"][c * P:(c + 1) * P, :])
            nc.sync.dma_start(out=b1buf[:], in_=d_b1[:])
            nc.sync.dma_start(out=ones1[:], in_=d_on[:])
            for c in range(NCH):
                nc.sync.dma_start(out=k1hi[:, c, :], in_=dw["k1hi"][c * P:(c + 1) * P, :])
                nc.sync.dma_start(out=k1lo[:, c, :], in_=dw["k1lo"][c * P:(c + 1) * P, :])
            nc.vector.memset(shl[0][:], 0.0)
            nc.vector.memset(shl[1][:, :, B:2 * B], 0.0)
            nc.vector.memset(hblo[:], 0.0)

            # ---- projection helper: alternate psum buffers across d so the
            # ACT drain of one block never shares a bank with the next
            # block's matmuls ----
            def _proj_segs(d, ncols):
                n3 = ncols - 1024
                if d % 2 == 0:
                    return [(0, 512, ps_x[:, 0:512]),
                            (512, 512, ps_x[:, 512:1024]),
                            (1024, n3, ps_s[:, 6, 0:n3])]
                return [(0, 512, ps_s[:, 0:2, :]),
                        (512, 512, ps_s[:, 2:4, :]),
                        (1024, n3, ps_s[:, 4, 0:n3])]

            # ---- P0: X0x = K0aug.T @ u_aug  -> xbuf (fp32) ----
            for d in range(NCH):
                for (o, n, sl) in _proj_segs(d, X0C):
                    nc.tensor.matmul(
                        sl,
                        k0buf[:, d * P:(d + 1) * P],
                        uin[:, o:o + n],
                        start=True, stop=True)
                    nc.scalar.activation(xbuf[:, d, o:o + n], sl, AF.Copy)

            # ---- scan step (shared pipeline skeleton) ----
            # Stagger over GROUPS: emit MM(G[k]), add(G[k-1]), stt(G[k-2]),
            # post(G[k-3]); the adds run as soon as their group's matmuls
            # retire (own PSUM bank), the blend chain of the last single-
            # chunk groups finishes right behind the final matmuls.
            def run_step(mm_group, add_g, stt_g, post_g):
                ng = len(GROUPS)
                for k in range(ng + 3):
                    if k < ng:
                        mm_group(*GROUPS[k])
                    if 0 <= k - 1 < ng:
                        add_g(*GROUPS[k - 1])
                    if 0 <= k - 2 < ng:
                        stt_g(*GROUPS[k - 2])
                    if 0 <= k - 3 < ng and post_g is not None:
                        post_g(*GROUPS[k - 3])

            # ---- cheap (bf16) scan step ----
            def cheap_step(par, xb, rb=None):
                # xb: x col base (int); rb: record col base or None
                si, so = shl[par], shl[1 - par]

                def mm_group(g, gn):
                    for d in range(g, g + gn):
                        for c in range(NCH):
                            nc.tensor.matmul(
                                _psl(d), whi[:, c, d * P:(d + 1) * P],
                                si[:, c, 0:B],
                                start=(c == 0), stop=(c == NCH - 1))

                def add_g(g, gn):
                    gs = slice(g, g + gn)
                    nc.vector.tensor_tensor(
                        out=zg[:, gs, :], in0=_psg(g, gn),
                        in1=xbuf[:, gs, ds(xb, B, 1)], op=OP.add)
                    nc.scalar.activation(gt[:, gs, :], zg[:, gs, :], AF.Tanh)

                def stt_g(g, gn):
                    gs = slice(g, g + gn)
                    nc.vector.scalar_tensor_tensor(
                        out=so[:, gs, 0:B], in0=si[:, gs, 0:B], scalar=0.5,
                        in1=gt[:, gs, :], op0=OP.mult, op1=OP.add)

                run_step(mm_group, add_g, stt_g, None)
                # records go last: they aren't read until P2, and issuing
                # them inside the pipeline delays the critical tanh chain
                # in the ACT FIFO
                if rb is not None:
                    nc.scalar.activation(hbhi[:, :, rb:rb + B],
                                         so[:, :, 0:B], AF.Copy)

            # ---- precise (split3) scan step ----
            def prec_step(i, par, mod, first=False, out_i=None):
                hi_m, ho_m = hm[par], hm[1 - par]
                si, so = shl[par], shl[1 - par]
                xb = _x0base(i) if mod == 0 else _x1base(i)
                rb = _recbase(i) if mod == 0 else None
                if first:
                    # master state = fp32 copy of the bf16 cheap state
                    nc.vector.tensor_copy(out=hi_m[:], in_=si[:, :, 0:B])

                def mm_group(g, gn):
                    for d in range(g, g + gn):
                        psl = _psl(d)
                        for c in range(NCH):
                            wsl = (slice(None), c, slice(d * P, (d + 1) * P))
                            nc.tensor.matmul(psl, whi[wsl], si[:, c, 0:B],
                                             start=(c == 0), stop=False)
                            nc.tensor.matmul(psl, whi[wsl], si[:, c, B:2 * B],
                                             start=False, stop=False)
                            nc.tensor.matmul(psl, wlo[wsl], si[:, c, 0:B],
                                             start=False, stop=(c == NCH - 1))

                def add_g(g, gn):
                    gs = slice(g, g + gn)
                    nc.vector.tensor_tensor(
                        out=zg[:, gs, :], in0=_psg(g, gn),
                        in1=xbuf[:, gs, xb:xb + B], op=OP.add)
                    nc.scalar.activation(gt[:, gs, :], zg[:, gs, :], AF.Tanh)

                def stt_g(g, gn):
                    gs = slice(g, g + gn)
                    nc.vector.scalar_tensor_tensor(
                        out=ho_m[:, gs, :], in0=hi_m[:, gs, :], scalar=0.5,
                        in1=gt[:, gs, :], op0=OP.mult, op1=OP.add)
                    # hi half of the bf16 split (lo follows in post)
                    nc.vector.tensor_copy(out=so[:, gs, 0:B], in_=ho_m[:, gs, :])

                def post_g(g, gn):
                    gs = slice(g, g + gn)
                    nc.vector.tensor_tensor(out=so[:, gs, B:2 * B],
                                            in0=ho_m[:, gs, :], in1=so[:, gs, 0:B],
                                            op=OP.subtract)

                run_step(mm_group, add_g, stt_g, post_g)
                # records last (not read until P2); output scale on the DVE
                # whose queue drains before the next step's first add anyway
                if rb is not None:
                    nc.scalar.activation(hbhi[:, :, rb:rb + B],
                                         so[:, :, 0:B], AF.Copy)
                    nc.scalar.activation(hblo[:, :, rb:rb + B],
                                         so[:, :, B:2 * B], AF.Copy)
                if out_i is not None:
                    nc.vector.tensor_scalar_mul(hout[:], ho_m[:], 0.5)
                    dst = d_out0 if mod == 0 else d_out1
                    nc.sync.dma_start(out=dst[out_i], in_=hout[:])

            # ---- P1: module-0 scan ----
            for i in range(0, W0T - W_WASH):
                cheap_step(i % 2, _x0base(i))
            for i in range(W0T - W_WASH, S0 - NPREC):
                cheap_step(i % 2, _x0base(i), _recbase(i))
            for i in range(S0 - NPREC, S0):
                prec_step(i, i % 2, 0, first=(i == S0 - NPREC),
                          out_i=(i - (S0 - L) if i >= S0 - L else None))

            # ---- load W1 into whi/wlo (after P1's last use) ----
            for c in range(NCH):
                nc.sync.dma_start(out=whi[:, c, :], in_=dw["w1hi"][c * P:(c + 1) * P, :])
                nc.sync.dma_start(out=wlo[:, c, :], in_=dw["w1lo"][c * P:(c + 1) * P, :])

            # ---- P2: X1x = K1h.T @ s0 + b1 (ones row) -> xbuf ----
            # x1 and hb share the phase-major layout, so moving cols = psum cols
            for d in range(NCH):
                segs = _proj_segs(d, X1C)
                for c in range(NCH):
                    for (o, n, psl) in segs:
                        ksl = (slice(None), c, slice(d * P, (d + 1) * P))
                        nc.tensor.matmul(psl, k1hi[ksl], hbhi[:, c, o:o + n],
                                         start=(c == 0), stop=False)
                        nc.tensor.matmul(psl, k1hi[ksl], hblo[:, c, o:o + n],
                                         start=False, stop=False)
                        nc.tensor.matmul(psl, k1lo[ksl], hbhi[:, c, o:o + n],
                                         start=False, stop=False)
                for (o, n, psl) in segs:
                    nc.tensor.matmul(
                        psl,
                        b1buf[:, d * P:(d + 1) * P],
                        ones1[:, o:o + n],
                        start=False, stop=True)
                    nc.scalar.activation(xbuf[:, d, o:o + n], psl, AF.Copy)

            # reset scan state for module 1 (hi of par 0 and stale lo of both)
            nc.vector.memset(shl[0][:], 0.0)
            nc.vector.memset(shl[1][:, :, B:2 * B], 0.0)

            # ---- P3: module-1 scan ----
            for j in range(0, S1 - NPREC):
                cheap_step(j % 2, _x1base(j))
            for j in range(S1 - NPREC, S1):
                prec_step(j, j % 2, 1, first=(j == S1 - NPREC),
                          out_i=(j - (S1 - L) if j >= S1 - L else None))

    nc.compile()
    return nc


def _bf16_pair(x):
    import ml_dtypes
    hi = x.astype(ml_dtypes.bfloat16)
    lo = (x - hi.astype(np.float32)).astype(ml_dtypes.bfloat16)
    return hi, lo


def _host_inputs(u, kernel0, rec0, bias0, kernel1, rec1, bias1):
    u = np.asarray(u, dtype=np.float32).reshape(T, IN)
    w0hi, w0lo = _bf16_pair(0.5 * np.asarray(rec0, dtype=np.float32))
    w1hi, w1lo = _bf16_pair(0.5 * np.asarray(rec1, dtype=np.float32))
    k1hi, k1lo = _bf16_pair(0.5 * np.asarray(kernel1, dtype=np.float32))
    k0aug = np.concatenate(
        [np.asarray(kernel0, dtype=np.float32),
         np.asarray(bias0, dtype=np.float32).reshape(1, UNITS)], axis=0)
    b1row = np.asarray(bias1, dtype=np.float32).reshape(1, UNITS).copy()

    # phase-major column maps: x0 col (ph, sig) <-> t = L*sig + ph
    ph0, sg0 = np.meshgrid(np.arange(L), np.arange(-PAD0, B), indexing="ij")
    t0map = (L * sg0 + ph0).reshape(-1)          # x0 col -> core-relative time
    ph1, sg1 = np.meshgrid(np.arange(L), np.arange(-PAD1, B), indexing="ij")
    t1map = (L * sg1 + ph1).reshape(-1)

    in_maps = []
    for core in range(NCORES):
        s0 = core * SPAN
        tg = s0 + t0map                          # global times per x0 col
        u_aug = np.zeros((IN + 1, X0C), dtype=np.float32)
        ok = tg >= 0
        u_aug[:IN, ok] = u[tg[ok]].T
        u_aug[IN, ok] = 1.0
        ones1 = np.zeros((1, X1C), dtype=np.float32)
        ones1[0, (s0 + t1map) >= 0] = 1.0
        in_maps.append({
            "w0hi": w0hi, "w0lo": w0lo, "w1hi": w1hi, "w1lo": w1lo,
            "k1hi": k1hi, "k1lo": k1lo, "k0aug": k0aug,
            "b1row": b1row, "u_aug": u_aug, "ones1": ones1,
        })
    return in_maps


def _reorder(arr):
    # arr [L, P, NCH*B] -> [SPAN, UNITS]; element (i, p, c*B+s) is
    # h at (row s*L+i, col c*P+p)
    a = arr.reshape(L, P, NCH, B)
    return a.transpose(3, 0, 2, 1).reshape(SPAN, UNITS)


def kernel(u, kernel0, rec0, bias0, kernel1, rec1, bias1):
    if "nc" not in _CACHE:
        _CACHE["nc"] = _build()
    nc = _CACHE["nc"]
    in_maps = _host_inputs(u, kernel0, rec0, bias0, kernel1, rec1, bias1)
    res = run_bass_kernel_spmd(nc, in_maps, core_ids=list(range(NCORES)))
    out = np.empty((T, 2 * UNITS), dtype=np.float32)
    for c in range(NCORES):
        out[c * SPAN:(c + 1) * SPAN, :UNITS] = _reorder(res.results[c]["out0"])
        out[c * SPAN:(c + 1) * SPAN, UNITS:] = _reorder(res.results[c]["out1"])
    return out.reshape(1, T, 2 * UNITS)
